# revision 1
# baseline (speedup 1.0000x reference)
"""Self-contained Trainium2 Bass kernel for nn_BRC_62715112457019 (sparse_attention).

kernel(**inputs) takes the FULL unsharded inputs (F, P, norm_weight, norm_bias),
shards head-parallel across 8 NeuronCores (core c computes attention head c for
both batch samples), runs the Bass/Tile program via run_bass_kernel_spmd, and
gathers the per-core outputs into the full (2, 64, 48, 48) float32 result.
"""
import sys
for _p in ('/opt/trn_rl_repo', '/opt/pypackages'):
    if _p not in sys.path:
        sys.path.insert(0, _p)
import numpy as np
import ml_dtypes
from contextlib import ExitStack

import concourse.bass as bass
import concourse.bacc as bacc
import concourse.tile as tile
from concourse import mybir

dt = mybir.dt
F32 = dt.float32
BF16 = dt.bfloat16
AF = mybir.ActivationFunctionType
OP = mybir.AluOpType

HW = 2304
CHUNKS = [(0, 512), (512, 512), (1024, 512), (1536, 512), (2048, 256)]
JCH = [(256 * i, 256) for i in range(9)]          # phase-B query chunks
KQUADS = [(0, 4), (4, 4), (8, 4), (12, 4), (16, 2)]  # kk-block groups (quad row-tiled)
NKB = 18           # 128-wide kk blocks
GRP = 3            # kk blocks per exp group
NGRP = NKB // GRP
TINR = 104         # TIN rows: [0:8) v=Fn, [8:64) ones, [64:72) qc, [72:96) ones, [96:104) kc
TRW = 104          # trT cols per kk block (transposed TIN chunk)
NEG = -30.0        # masked-key logit bias
BF = ml_dtypes.bfloat16


def host_constants(w8, b8):
    eye = np.eye(128, dtype=np.float32)
    # batched per-chunk selectors: sample 0 -> out rows 0:5, sample 1 -> rows 32:37
    selsum = np.zeros((128, 5 * 37), BF)
    for c in range(5):
        selsum[0:64, 37 * c + c] = 1.0
        selsum[64:128, 37 * c + 32 + c] = 1.0
    selq = np.zeros((16, 5 * 37), np.float32)
    for c in range(5):
        selq[0:8, 37 * c + c] = 1.0
        selq[8:16, 37 * c + 32 + c] = 1.0
    wb16 = np.zeros((16, 2), np.float32)
    wb16[0:8, 0] = w8
    wb16[8:16, 0] = w8
    wb16[0:8, 1] = b8
    wb16[8:16, 1] = b8
    return {"eye": eye, "selsum": selsum, "wb16": wb16, "selq": selq}


def make_inmaps(F, P, norm_weight, norm_bias):
    F = np.asarray(F, np.float32).reshape(2, 64, HW)
    P = np.asarray(P, np.float32).reshape(2, 48, 48)
    w = np.asarray(norm_weight, np.float32)
    b = np.asarray(norm_bias, np.float32)
    maps = []
    for c in range(8):
        m = host_constants(w[8 * c:8 * c + 8], b[8 * c:8 * c + 8])
        for n in range(2):
            m[f"Fb{n}"] = np.ascontiguousarray(F[n].astype(BF))
            m[f"F8_{n}"] = np.ascontiguousarray(F[n, 8 * c:8 * c + 8])
            m[f"P{n}"] = np.ascontiguousarray(P[n])
        maps.append(m)
    return maps


def assemble(results):
    out = np.empty((2, 64, 48, 48), np.float32)
    for c in range(8):
        for n in range(2):
            out[n, 8 * c:8 * c + 8] = results[c][f"out{n}"].reshape(8, 48, 48)
    return out


def build_program():
    nc = bacc.Bacc("TRN2", target_bir_lowering=False, debug=False)
    ins = {}
    for n in range(2):
        ins[f"Fb{n}"] = nc.dram_tensor(f"Fb{n}", [64, HW], BF16, kind="ExternalInput").ap()
        ins[f"F8_{n}"] = nc.dram_tensor(f"F8_{n}", [8, HW], F32, kind="ExternalInput").ap()
        ins[f"P{n}"] = nc.dram_tensor(f"P{n}", [48, 48], F32, kind="ExternalInput").ap()
    ins["eye"] = nc.dram_tensor("eye", [128, 128], F32, kind="ExternalInput").ap()
    ins["selsum"] = nc.dram_tensor("selsum", [128, 185], BF16, kind="ExternalInput").ap()
    ins["wb16"] = nc.dram_tensor("wb16", [16, 2], F32, kind="ExternalInput").ap()
    ins["selq"] = nc.dram_tensor("selq", [16, 185], F32, kind="ExternalInput").ap()
    outs = [nc.dram_tensor(f"out{n}", [8, HW], F32, kind="ExternalOutput").ap() for n in range(2)]

    with tile.TileContext(nc) as tc:
        with ExitStack() as ctx:
            _body(ctx, tc, nc, ins, outs)
    nc.compile()
    return nc


# sob master sub-tile slots (pairs of 50 cols: sample0|sample1, rows 0:48)
S_P50, S_PM, S_A1, S_TMP, S_B1, S_A1P, S_B1P, S_TCOL, S_GXT, S_GYT, S_M1, S_M2, \
    S_STT, S_BTM, S_BHW, S_FG, S_BG, S_BB = range(18)


def _body(ctx, tc, nc, ins, outs):
    pers = ctx.enter_context(tc.tile_pool(name="pers", bufs=1))
    big = ctx.enter_context(tc.tile_pool(name="big", bufs=7))
    sm = ctx.enter_context(tc.tile_pool(name="sm", bufs=1))

    eye = pers.tile([128, 128], F32, tag="eye")
    nc.sync.dma_start(eye[:], ins["eye"])
    selsum = pers.tile([128, 185], BF16, tag="selsum")
    nc.sync.dma_start(selsum[:], ins["selsum"])
    wb16 = pers.tile([16, 2], F32, tag="wb16")
    nc.sync.dma_start(wb16[:], ins["wb16"])
    selq = pers.tile([16, 185], F32, tag="selq")
    nc.sync.dma_start(selq[:], ins["selq"])
    consts = pers.tile([128, 2], F32, tag="consts")   # col0 = eps
    nc.vector.memset(consts[:, 0:1], 1e-5)
    bch = []
    for n in range(2):
        t = pers.tile([5, 512], F32, tag=f"bch{n}", name=f"bch{n}")
        nc.vector.memset(t[:], 0.0)
        bch.append(t)

    onesrow = pers.tile([1, HW], BF16, tag="onesrow")
    nc.vector.memset(onesrow[:], 1.0)
    TIN, trT, QBl, QBr = [], [], [], []
    for n in range(2):
        t = pers.tile([TINR, HW], F32, tag=f"TIN{n}", name=f"TIN{n}")
        nc.gpsimd.memset(t[:], 1.0)
        TIN.append(t)
        trT.append(pers.tile([128, NKB * TRW], BF16, tag=f"trT{n}", name=f"trT{n}"))
        ql = pers.tile([9, HW], BF16, tag=f"QBl{n}", name=f"QBl{n}")
        nc.gpsimd.memset(ql[:], 0.0)
        QBl.append(ql)
        qr = pers.tile([9, HW], BF16, tag=f"QBr{n}", name=f"QBr{n}")
        nc.gpsimd.memset(qr[:], 0.0)
        QBr.append(qr)
    B3b = pers.tile([16, HW], F32, tag="B3b")    # batched; rows 0:8 = sample0

    # =============== Phase A: batched over both samples ===============
    with tc.tile_pool(name="psA", bufs=4, space="PSUM") as psA:

        def pbank(nm):
            return psA.tile([128, 512], F32, tag="pbank", name=nm)

        # ---- LayerNorm stats (both samples via 128-row stack) ----
        F128 = big.tile([128, HW], BF16, tag="big", name="F128")
        for off, w in CHUNKS:
            nc.sync.dma_start(F128[0:64, off:off + w], ins["Fb0"][:, off:off + w])
            nc.sync.dma_start(F128[64:128, off:off + w], ins["Fb1"][:, off:off + w])
        Fsq = big.tile([128, HW], BF16, tag="big", name="Fsq")
        for off, w in CHUNKS:
            nc.vector.tensor_tensor(Fsq[:, off:off + w], F128[:, off:off + w],
                                    F128[:, off:off + w], OP.mult)
        F16 = big.tile([16, HW], F32, tag="big", name="F16")
        nc.sync.dma_start(F16[0:8, :], ins["F8_0"])
        nc.sync.dma_start(F16[8:16, :], ins["F8_1"])

        psumsA = pbank("psumsA")
        psumsB = pbank("psumsB")
        for c, (off, w) in enumerate(CHUNKS):
            nc.tensor.matmul(psumsA[0:37, 0:w], selsum[:, 37 * c:37 * c + 37],
                             F128[:, off:off + w], start=(c == 0), stop=(c == 4))
            nc.tensor.matmul(psumsB[0:37, 0:w], selsum[:, 37 * c:37 * c + 37],
                             Fsq[:, off:off + w], start=(c == 0), stop=(c == 4))
        # stats on (37,512): rows 0:5 = sample0 chunks, 32:37 = sample1
        stm = sm.tile([37, 2560], F32, tag="stm")
        s2 = stm[:, 0:512]
        varT = stm[:, 512:1024]
        sd = stm[:, 1024:1536]
        rstd = stm[:, 1536:2048]
        mu = stm[:, 2048:2560]
        nc.scalar.activation(s2, psumsA[0:37, :], AF.Square, scale=0.125)
        nc.vector.scalar_tensor_tensor(varT, psumsB[0:37, :], 1.0, s2, OP.mult, OP.subtract)
        nc.scalar.activation(sd, varT, AF.Sqrt, bias=consts[0:37, 0:1], scale=1.0 / 64.0)
        nc.vector.reciprocal(rstd, sd)
        nc.vector.tensor_scalar(mu, psumsA[0:37, :], 1.0 / 64.0, None, OP.mult)
        murow = big.tile([2, HW], F32, tag="big", name="murow")
        rsrow = big.tile([2, HW], F32, tag="big", name="rsrow")
        for r, lo in ((0, 0), (1, 32)):
            nc.sync.dma_start(murow[r:r + 1, 0:2048], mu[lo:lo + 4, :])
            nc.sync.dma_start(murow[r:r + 1, 2048:2304], mu[lo + 4:lo + 5, 0:256])
            nc.sync.dma_start(rsrow[r:r + 1, 0:2048], rstd[lo:lo + 4, :])
            nc.sync.dma_start(rsrow[r:r + 1, 2048:2304], rstd[lo + 4:lo + 5, 0:256])
        mu16 = big.tile([16, HW], F32, tag="big", name="mu16")
        rs16 = big.tile([16, HW], F32, tag="big", name="rs16")
        for r in range(2):
            nc.sync.dma_start(mu16[8 * r:8 * r + 8, :],
                              murow[r:r + 1, :].unsqueeze(1).broadcast_to([1, 8, HW]))
            nc.sync.dma_start(rs16[8 * r:8 * r + 8, :],
                              rsrow[r:r + 1, :].unsqueeze(1).broadcast_to([1, 8, HW]))
        dtmp = big.tile([16, HW], F32, tag="big", name="dtmp")
        nc.vector.tensor_tensor(dtmp[:], F16[:], mu16[:], OP.subtract)
        nc.vector.tensor_tensor(dtmp[:], dtmp[:], rs16[:], OP.mult)
        Fnb = pers.tile([16, HW], F32, tag="Fnb")
        nc.vector.tensor_scalar(Fnb[:], dtmp[:], wb16[:, 0:1], wb16[:, 1:2], OP.mult, OP.add)
        nc.vector.tensor_copy(TIN[0][0:8, :], Fnb[0:8, :])
        nc.sync.dma_start(TIN[1][0:8, :], Fnb[8:16, :])

        # ---- masks, batched in the free dim (sample slots side by side) ----
        sobm = sm.tile([48, 100 * 18], F32, tag="sobm")
        sv = sobm[:].rearrange("p (i s c) -> p i s c", s=2, c=50)

        def slot(i, r=(1, 49)):
            return sv[:, i, :, r[0]:r[1]]

        nc.gpsimd.memset(sobm[:, 0:200], 0.0)
        nc.sync.dma_start(slot(S_P50)[:, 0, :], ins["P0"])
        nc.sync.dma_start(slot(S_P50)[:, 1, :], ins["P1"])
        nc.scalar.activation(slot(S_PM), slot(S_P50), AF.Sigmoid)
        Pm0 = sv[:, S_PM]
        nc.vector.tensor_tensor(slot(S_A1), Pm0[:, :, 0:48], Pm0[:, :, 2:50], OP.subtract)
        nc.vector.tensor_tensor(slot(S_TMP), Pm0[:, :, 0:48], Pm0[:, :, 2:50], OP.add)
        nc.vector.scalar_tensor_tensor(slot(S_B1), Pm0[:, :, 1:49], 2.0, slot(S_TMP),
                                       OP.mult, OP.add)
        nc.gpsimd.memset(sobm[:, 100 * S_A1P:100 * S_A1P + 200], 0.0)  # A1P+B1P pads
        for s in range(2):
            pt1 = pbank(f"pt1_{s}")
            nc.tensor.transpose(pt1[0:48, 0:48], slot(S_A1)[:, s, :], eye[0:48, 0:48])
            nc.vector.tensor_copy(slot(S_A1P)[:, s, :], pt1[0:48, 0:48])
            pt2 = pbank(f"pt2_{s}")
            nc.tensor.transpose(pt2[0:48, 0:48], slot(S_B1)[:, s, :], eye[0:48, 0:48])
            nc.vector.tensor_copy(slot(S_B1P)[:, s, :], pt2[0:48, 0:48])
        A1p = sv[:, S_A1P]
        B1p = sv[:, S_B1P]
        nc.vector.tensor_tensor(slot(S_TCOL), A1p[:, :, 0:48], A1p[:, :, 2:50], OP.add)
        nc.vector.scalar_tensor_tensor(slot(S_GXT), A1p[:, :, 1:49], 2.0, slot(S_TCOL),
                                       OP.mult, OP.add)
        nc.vector.tensor_tensor(slot(S_GYT), B1p[:, :, 0:48], B1p[:, :, 2:50], OP.subtract)
        nc.vector.tensor_tensor(slot(S_M1), slot(S_GXT), slot(S_GXT), OP.mult)
        nc.vector.tensor_tensor(slot(S_M2), slot(S_GYT), slot(S_GYT), OP.mult)
        nc.vector.tensor_tensor(slot(S_STT), slot(S_M1), slot(S_M2), OP.add)
        nc.vector.tensor_scalar(slot(S_BTM), slot(S_STT), 0.0, None, OP.is_gt)
        for s in range(2):
            pt3 = pbank(f"pt3_{s}")
            nc.tensor.transpose(pt3[0:48, 0:48], slot(S_BTM)[:, s, :], eye[0:48, 0:48])
            nc.vector.tensor_copy(slot(S_BHW)[:, s, :], pt3[0:48, 0:48])
        nc.vector.tensor_scalar(slot(S_FG), slot(S_P50), 0.0, None, OP.is_gt)
        nc.vector.tensor_scalar(slot(S_BG), slot(S_P50), 0.0, None, OP.is_lt)
        nc.vector.scalar_tensor_tensor(slot(S_BB), slot(S_BG), 1.0, slot(S_BHW),
                                       OP.mult, OP.max)
        fgrow = big.tile([2, HW], F32, tag="big", name="fgrow")
        bbrow = big.tile([2, HW], F32, tag="big", name="bbrow")
        brow = big.tile([2, HW], F32, tag="big", name="brow")
        for s in range(2):
            nc.sync.dma_start(fgrow[s:s + 1, :], slot(S_FG)[:, s, :])
            nc.sync.dma_start(bbrow[s:s + 1, :], slot(S_BB)[:, s, :])
            nc.sync.dma_start(brow[s:s + 1, :], slot(S_BHW)[:, s, :])
            nc.sync.dma_start(bch[s][0:4, 0:512], brow[s:s + 1, 0:2048])
            nc.sync.dma_start(bch[s][4:5, 0:256], brow[s:s + 1, 2048:2304])
        biasr = sm.tile([2, HW], BF16, tag="biasr")
        nc.vector.tensor_scalar(biasr[:], fgrow[:], -NEG, NEG, OP.mult, OP.add)
        nc.sync.dma_start(QBl[0][8:9, :], biasr[0:1, :])
        nc.sync.dma_start(QBl[1][8:9, :], biasr[1:2, :])

        # ---- masked features + channel norms (batched 16 rows) ----
        fg16 = big.tile([16, HW], F32, tag="big", name="fg16")
        bbg16 = big.tile([16, HW], F32, tag="big", name="bbg16")
        b16 = big.tile([16, HW], F32, tag="big", name="b16")
        for r in range(2):
            nc.sync.dma_start(fg16[8 * r:8 * r + 8, :],
                              fgrow[r:r + 1, :].unsqueeze(1).broadcast_to([1, 8, HW]))
            nc.sync.dma_start(bbg16[8 * r:8 * r + 8, :],
                              bbrow[r:r + 1, :].unsqueeze(1).broadcast_to([1, 8, HW]))
            nc.sync.dma_start(b16[8 * r:8 * r + 8, :],
                              brow[r:r + 1, :].unsqueeze(1).broadcast_to([1, 8, HW]))
        fgf = pers.tile([16, HW], F32, tag="fgf")
        nc.vector.tensor_tensor(fgf[:], Fnb[:], fg16[:], OP.mult)
        bbgf = big.tile([16, HW], F32, tag="big", name="bbgf")
        nc.vector.tensor_tensor(bbgf[:], Fnb[:], bbg16[:], OP.mult)
        smmst = sm.tile([16, 48], F32, tag="smmst")
        nc.vector.memset(smmst[:], 0.0)
        sqf = big.tile([16, HW], F32, tag="big", name="sqf")
        nc.vector.scalar_tensor_tensor(sqf[:], fgf[:], 1.0, fgf[:], OP.mult, OP.mult,
                                       accum_out=smmst[:, 0:1])
        nc.scalar.activation(smmst[:, 1:2], smmst[:, 0:1], AF.Sqrt)
        nc.vector.tensor_scalar(smmst[:, 1:2], smmst[:, 1:2], 1e-12, None, OP.max)
        nc.vector.reciprocal(smmst[:, 2:3], smmst[:, 1:2])
        kc16 = big.tile([16, HW], F32, tag="big", name="kc16")
        nc.vector.tensor_scalar(kc16[:], fgf[:], smmst[:, 2:3], None, OP.mult)
        nc.sync.dma_start(TIN[0][96:104, :], kc16[0:8, :])
        nc.sync.dma_start(TIN[1][96:104, :], kc16[8:16, :])
        sqb = big.tile([16, HW], F32, tag="big", name="sqb")
        nc.vector.scalar_tensor_tensor(sqb[:], bbgf[:], 1.0, bbgf[:], OP.mult, OP.mult,
                                       accum_out=smmst[:, 3:4])
        nc.scalar.activation(smmst[:, 4:5], smmst[:, 3:4], AF.Sqrt)
        nc.vector.tensor_scalar(smmst[:, 4:5], smmst[:, 4:5], 1e-12, None, OP.max)
        nc.vector.reciprocal(smmst[:, 5:6], smmst[:, 4:5])
        qc16 = pers.tile([16, HW], F32, tag="qc16")
        nc.vector.tensor_scalar(qc16[:], bbgf[:], smmst[:, 5:6], None, OP.mult)
        nc.sync.dma_start(TIN[0][64:72, :], qc16[0:8, :])
        nc.sync.dma_start(TIN[1][64:72, :], qc16[8:16, :])

        # ---- spatial q (batched) ----
        sq16 = big.tile([16, HW], F32, tag="big", name="sq16")
        nc.vector.tensor_tensor(sq16[:], Fnb[:], Fnb[:], OP.mult)
        pssq = pbank("pssq")
        for c, (off, w) in enumerate(CHUNKS):
            nc.tensor.matmul(pssq[0:37, 0:w], selq[:, 37 * c:37 * c + 37],
                             sq16[:, off:off + w], start=(c == 0), stop=(c == 4))
        sqs = sm.tile([37, 1024], F32, tag="sqs")
        nc.scalar.activation(sqs[:, 0:512], pssq[0:37, :], AF.Sqrt)
        nc.vector.reciprocal(sqs[:, 512:1024], sqs[:, 0:512])
        rqrow = big.tile([2, HW], F32, tag="big", name="rqrow")
        for r, lo in ((0, 0), (1, 32)):
            nc.sync.dma_start(rqrow[r:r + 1, 0:2048], sqs[lo:lo + 4, 512:1024])
            nc.sync.dma_start(rqrow[r:r + 1, 2048:2304], sqs[lo + 4:lo + 5, 512:768])
        rq16 = big.tile([16, HW], F32, tag="big", name="rq16")
        for r in range(2):
            nc.sync.dma_start(rq16[8 * r:8 * r + 8, :],
                              rqrow[r:r + 1, :].unsqueeze(1).broadcast_to([1, 8, HW]))
        q16 = big.tile([16, HW], F32, tag="big", name="q16")
        nc.vector.tensor_tensor(q16[:], Fnb[:], rq16[:], OP.mult)
        qcast = big.tile([16, HW], BF16, tag="big", name="qcast")
        nc.vector.tensor_copy(qcast[:], q16[:])
        nc.vector.tensor_copy(QBl[0][0:8, :], qcast[0:8, :])
        nc.sync.dma_start(QBl[1][0:8, :], qcast[8:16, :])
        for n in range(2):
            nc.sync.dma_start(QBr[n][0:8, :], QBl[n][0:8, :])
            nc.sync.dma_start(QBr[n][8:9, :], onesrow[:])

        # ---- B3 = 2*Fn + b*(q - Fn) (batched; split after Fch) ----
        nc.vector.tensor_tensor(B3b[:], q16[:], Fnb[:], OP.subtract)
        nc.vector.tensor_tensor(B3b[:], B3b[:], b16[:], OP.mult)
        nc.vector.scalar_tensor_tensor(B3b[:], Fnb[:], 2.0, B3b[:], OP.mult, OP.add)

        # ---- per-sample transposes + channel-attn logits ----
        lcP = []
        for n in range(2):
            plcT = pbank(f"plc{n}")
            for b in range(NKB):
                pt = pbank(f"ptr{n}_{b}")
                nc.tensor.transpose(pt[:, 0:TINR], TIN[n][:, 128 * b:128 * (b + 1)],
                                    eye[0:TINR, 0:TINR])
                nc.vector.tensor_copy(trT[n][:, TRW * b:TRW * b + TRW], pt[:, 0:TINR])
                nc.tensor.matmul(plcT[0:8, 0:8], trT[n][:, TRW * b + 64:TRW * b + 72],
                                 trT[n][:, TRW * b + 96:TRW * b + 104],
                                 start=(b == 0), stop=(b == NKB - 1))
            lcP.append(plcT)

        # ---- channel attention AV (block-diag batched matmul) ----
        lcf = sm.tile([16, 16], F32, tag="lcf")
        nc.gpsimd.memset(lcf[:], 0.0)
        rs16v = sm.tile([16, 2], F32, tag="rs16v")
        for n in range(2):
            nc.vector.tensor_copy(smmst[0:8, 8 + 8 * n:16 + 8 * n], lcP[n][0:8, 0:8])
        exp0 = smmst[0:8, 24:32]
        nc.scalar.activation(exp0, smmst[0:8, 8:16], AF.Exp, accum_out=rs16v[0:8, 0:1])
        exp1 = smmst[0:8, 32:40]
        nc.scalar.activation(exp1, smmst[0:8, 16:24], AF.Exp, accum_out=rs16v[0:8, 1:2])
        # rows 0:8 of rs16v col0 = sample0 sums; col1 rows 0:8 = sample1 sums
        nc.sync.dma_start(rs16v[8:16, 0:1], rs16v[0:8, 1:2])
        nc.vector.reciprocal(rs16v[:, 0:1], rs16v[:, 0:1])
        pex0 = pbank("pex0")
        nc.tensor.transpose(pex0[0:8, 0:8], exp0, eye[0:8, 0:8])
        nc.vector.tensor_copy(lcf[0:8, 0:8], pex0[0:8, 0:8])
        pex1 = pbank("pex1")
        nc.tensor.transpose(pex1[0:8, 0:8], exp1, eye[0:8, 0:8])
        lct1 = sm.tile([8, 8], F32, tag="lct1")
        nc.vector.tensor_copy(lct1[:], pex1[0:8, 0:8])
        nc.sync.dma_start(lcf[8:16, 8:16], lct1[:])
        Fch = big.tile([16, HW], F32, tag="big", name="Fch")
        for c, (off, w) in enumerate(CHUNKS):
            pfc = pbank(f"pfc{c}")
            nc.tensor.matmul(pfc[0:16, 0:w], lcf[:], fgf[:, off:off + w],
                             start=True, stop=True)
            nc.vector.scalar_tensor_tensor(Fch[:, off:off + w], pfc[0:16, 0:w],
                                           rs16v[:, 0:1], qc16[:, off:off + w],
                                           OP.mult, OP.add)
        nc.vector.tensor_tensor(B3b[:], B3b[:], Fch[:], OP.add)

    # =============== Phase B: spatial attention (flash over kk) ===============
    with tc.tile_pool(name="psL", bufs=2, space="PSUM") as psL, \
         tc.tile_pool(name="psO", bufs=2, space="PSUM") as psO, \
         tc.tile_pool(name="sS", bufs=3) as sS, \
         tc.tile_pool(name="sB", bufs=2) as sB:
        tV16 = big.tile([16, HW], F32, tag="big", name="tV16")
        rcb16 = big.tile([16, HW], F32, tag="big", name="rcb16")
        for n in range(2):
            dn6 = sm.tile([5, 1024], F32, tag="dn6", name=f"dn6_{n}")
            nc.gpsimd.memset(dn6[:], 1.0)

            # software-pipelined emission: logits for group g+1 are issued
            # before the AV matmuls of group g, so PE never waits on ACT exp
            for jc, (joff, jw) in enumerate(CHUNKS):
                outT = psO.tile([48, 512], F32, tag="outT")
                Ss = []

                def emit_logits(g):
                    Lg = psL.tile([128, GRP * 512], F32, tag="L", name=f"L{n}_{jc}_{g}")
                    for i in range(GRP):
                        b = GRP * g + i
                        nc.tensor.matmul(Lg[:, i * jw:(i + 1) * jw],
                                         QBl[n][:, 128 * b:128 * (b + 1)],
                                         QBr[n][:, joff:joff + jw],
                                         start=True, stop=True)
                    Sg = sS.tile([128, GRP * 512], BF16, tag="S", name=f"S{n}_{jc}_{g}")
                    nc.scalar.activation(Sg[:, 0:GRP * jw], Lg[:, 0:GRP * jw], AF.Exp)
                    Ss.append(Sg)

                def emit_av(g):
                    Sg = Ss[g]
                    for i in range(GRP):
                        b = GRP * g + i
                        nc.tensor.matmul(outT[:, 0:jw],
                                         trT[n][:, TRW * b:TRW * b + 48],
                                         Sg[:, i * jw:(i + 1) * jw],
                                         start=(b == 0), stop=(b == NKB - 1))

                emit_logits(0)
                for g in range(NGRP):
                    if g + 1 < NGRP:
                        emit_logits(g + 1)
                    emit_av(g)
                dj = sB.tile([33, 512], F32, tag="dj")
                if n == 0:
                    nc.vector.tensor_copy(tV16[0:8, joff:joff + jw], outT[0:8, 0:jw])
                else:
                    nc.vector.tensor_copy(dj[0:8, 0:jw], outT[0:8, 0:jw])
                    nc.sync.dma_start(tV16[8:16, joff:joff + jw], dj[0:8, 0:jw])
                nc.vector.tensor_copy(dj[32:33, 0:jw], outT[32:33, 0:jw])
                nc.sync.dma_start(dn6[jc:jc + 1, 0:jw], dj[32:33, 0:jw])
            # batched reciprocal of all denominators; fold in the b mask
            nc.vector.reciprocal(dn6[:, 512:1024], dn6[:, 0:512])
            nc.vector.tensor_tensor(dn6[:, 512:1024], dn6[:, 512:1024], bch[n][:],
                                    OP.mult)
            rcrow = big.tile([1, HW], F32, tag="big", name=f"rcrow{n}")
            nc.sync.dma_start(rcrow[0:1, 0:2048], dn6[0:4, 512:1024])
            nc.sync.dma_start(rcrow[0:1, 2048:2304], dn6[4:5, 512:768])
            nc.sync.dma_start(rcb16[8 * n:8 * n + 8, :],
                              rcrow[0:1, :].unsqueeze(1).broadcast_to([1, 8, HW]))
        bt16 = big.tile([16, HW], F32, tag="big", name="bt16")
        nc.vector.tensor_tensor(bt16[:], tV16[:], rcb16[:], OP.mult)
        fin16 = big.tile([16, HW], F32, tag="big", name="fin16")
        nc.gpsimd.tensor_tensor(fin16[:], B3b[:], bt16[:], OP.add)
        nc.sync.dma_start(outs[0][:], fin16[0:8, :])
        nc.sync.dma_start(outs[1][:], fin16[8:16, :])


_PROGRAM = None


def _program():
    global _PROGRAM
    if _PROGRAM is None:
        _PROGRAM = build_program()
    return _PROGRAM


def kernel(F, P, norm_weight, norm_bias):
    from concourse.bass_utils import run_bass_kernel_spmd
    nc = _program()
    maps = make_inmaps(F, P, norm_weight, norm_bias)
    res = run_bass_kernel_spmd(nc, maps, core_ids=list(range(8)), trace=False)
    return assemble(res.results)



# revision 3
# speedup vs baseline: 1.1964x; 1.1964x over previous
"""Trainium2 Bass kernel for nn_BRC_62715112457019 (sparse_attention), v2.

Head-parallel across 8 cores (core c = head c, both samples). Pixel-major
phase A (per-pixel stats/masks live on partitions -> no broadcast DMAs, tiny
128-wide DVE ops), fp8 DoubleRow phase B (QK^T and AV at 2 fp8 MACs/cycle),
per-qchunk transposed epilogue (no row-broadcasts), overlapped channel-attn
path and output writeback.

Pixel blocking: block b in [0,18) covers pixels [128b, 128b+128). Pixel-major
tiles are [128, 18*K] with column group b. Channel-major tensors ([8|16, HW])
are produced/consumed via PE transposes per block.
"""
import sys
for _p in ('/opt/trn_rl_repo', '/opt/pypackages'):
    if _p not in sys.path:
        sys.path.insert(0, _p)
import numpy as np
import ml_dtypes
from contextlib import ExitStack

import concourse.bass as bass
import concourse.bacc as bacc
import concourse.tile as tile
from concourse import mybir

dt = mybir.dt
F32 = dt.float32
BF16 = dt.bfloat16
FP8 = dt.float8e4
AF = mybir.ActivationFunctionType
OP = mybir.AluOpType
DR = mybir.MatmulPerfMode.DoubleRow

HW = 2304
NB = 18                       # 128-pixel blocks
CHUNKS = [(0, 512), (512, 512), (1024, 512), (1536, 512), (2048, 256)]
GRP = 3                       # logit blocks per exp group
BF = ml_dtypes.bfloat16
F8 = ml_dtypes.float8_e4m3fn


def host_constants(w8, b8):
    eye = np.eye(128, dtype=np.float32)
    eyeb = np.eye(128, dtype=BF)
    selsum = np.zeros((128, 5 * 37), BF)
    for c in range(5):
        selsum[0:64, 37 * c + c] = 1.0
        selsum[64:128, 37 * c + 32 + c] = 1.0
    wb16 = np.zeros((16, 2), np.float32)
    wb16[0:8, 0] = w8
    wb16[8:16, 0] = w8
    wb16[0:8, 1] = b8
    wb16[8:16, 1] = b8
    ones16 = np.ones((16, 1), np.float32)
    return {"eye": eye, "eyeb": eyeb, "selsum": selsum, "wb16": wb16,
            "ones16": ones16}


def make_inmaps(F, P, norm_weight, norm_bias):
    F = np.asarray(F, np.float32).reshape(2, 64, HW)
    P = np.asarray(P, np.float32).reshape(2, 48, 48)
    w = np.asarray(norm_weight, np.float32)
    b = np.asarray(norm_bias, np.float32)
    maps = []
    for c in range(8):
        m = host_constants(w[8 * c:8 * c + 8], b[8 * c:8 * c + 8])
        for n in range(2):
            m[f"Fb{n}"] = np.ascontiguousarray(F[n].astype(BF))
            m[f"F8_{n}"] = np.ascontiguousarray(F[n, 8 * c:8 * c + 8])
            m[f"P{n}"] = np.ascontiguousarray(P[n])
        maps.append(m)
    return maps


def assemble(results):
    out = np.empty((2, 64, 48, 48), np.float32)
    for c in range(8):
        for n in range(2):
            out[n, 8 * c:8 * c + 8] = results[c][f"out{n}"].reshape(8, 48, 48)
    return out


def build_program(apply_wb):
    nc = bacc.Bacc("TRN2", target_bir_lowering=False, debug=False)
    ins = {}
    for n in range(2):
        ins[f"Fb{n}"] = nc.dram_tensor(f"Fb{n}", [64, HW], BF16, kind="ExternalInput").ap()
        ins[f"F8_{n}"] = nc.dram_tensor(f"F8_{n}", [8, HW], F32, kind="ExternalInput").ap()
        ins[f"P{n}"] = nc.dram_tensor(f"P{n}", [48, 48], F32, kind="ExternalInput").ap()
    ins["eye"] = nc.dram_tensor("eye", [128, 128], F32, kind="ExternalInput").ap()
    ins["eyeb"] = nc.dram_tensor("eyeb", [128, 128], BF16, kind="ExternalInput").ap()
    ins["selsum"] = nc.dram_tensor("selsum", [128, 185], BF16, kind="ExternalInput").ap()
    ins["wb16"] = nc.dram_tensor("wb16", [16, 2], F32, kind="ExternalInput").ap()
    ins["ones16"] = nc.dram_tensor("ones16", [16, 1], F32, kind="ExternalInput").ap()
    outs = [nc.dram_tensor(f"out{n}", [8, HW], F32, kind="ExternalOutput").ap() for n in range(2)]

    with tile.TileContext(nc) as tc:
        with ExitStack() as ctx:
            _body(ctx, tc, nc, ins, outs, apply_wb)
    nc.compile()
    return nc


# sobel slot indices (pairs of 50 cols: sample0|sample1, rows 0:48)
S_P50, S_PM, S_A1, S_TMP, S_B1, S_A1P, S_B1P, S_TCOL, S_GXT, S_GYT, S_M1, S_M2, \
    S_STT, S_BTM, S_BHW, S_FG, S_BG, S_BB = range(18)


def _body(ctx, tc, nc, ins, outs, apply_wb):
    pers = ctx.enter_context(tc.tile_pool(name="pers", bufs=1))
    sm = ctx.enter_context(tc.tile_pool(name="sm", bufs=1))

    # ---- persistent tiles ----
    eye = pers.tile([128, 128], F32, tag="eye")
    eyeb = pers.tile([128, 128], BF16, tag="eyeb")
    selsum = pers.tile([128, 185], BF16, tag="selsum")
    wb16 = pers.tile([16, 2], F32, tag="wb16")
    ones16 = pers.tile([16, 1], F32, tag="ones16")
    consts = pers.tile([128, 2], F32, tag="consts")     # col0 = eps
    F128 = pers.tile([128, HW], BF16, tag="F128")
    Fsq = pers.tile([128, HW], BF16, tag="Fsq")
    F16 = pers.tile([16, HW], F32, tag="F16")
    FnT = pers.tile([128, 288], F32, tag="FnT")         # 16b+8s+d
    qT = pers.tile([128, 288], F32, tag="qT")
    NRM = pers.tile([128, 36], F32, tag="NRM")          # 2b+s
    RQB = pers.tile([128, 72], F32, tag="RQB")          # [0:36] sqrt, [36:72] recip
    MT = pers.tile([128, 296], F32, tag="MT")           # 74j+37t+32s+c
    MKT = pers.tile([128, 108], F32, tag="MKT")         # 6b+{fg0,fg1,bb0,bb1,b0,b1}
    mrows = pers.tile([6, HW], F32, tag="mrows")
    qcm16 = pers.tile([16, HW], BF16, tag="qcm16")
    qcm1 = pers.tile([8, HW], BF16, tag="qcm1")
    trTav = [pers.tile([128, 288], FP8, tag=f"trTav{s}", name=f"trTav{s}") for s in range(2)]
    bfg = pers.tile([128, 864], F32, tag="bfg")        # 48b+24s+{fg8,bb8,Fn8}
    CM = [pers.tile([24, HW], BF16, tag=f"CM{s}", name=f"CM{s}") for s in range(2)]
    Sall = pers.tile([128, 2 * NB * 512], FP8, tag="Sall")
    w1 = pers.tile([128, 288], F32, tag="w1")
    spatT = pers.tile([128, 288], F32, tag="spatT")
    OUTT = pers.tile([128, 288], F32, tag="OUTT")
    fin = [pers.tile([8, HW], F32, tag=f"fin{s}", name=f"fin{s}") for s in range(2)]
    rc = pers.tile([128, 8], F32, tag="rc")             # epilogue denominators
    sobm = pers.tile([48, 100 * 18], F32, tag="sobm")
    stm = pers.tile([37, 2560], F32, tag="stm")
    sq = pers.tile([128, 16], F32, tag="sq")
    # channel path smalls
    msk = pers.tile([16, 32], F32, tag="msk")
    r16f = pers.tile([16, 4], F32, tag="r16f")          # [0:2] sqrt, [2:4]=1/max(sqrt,..); col s
    rqd = pers.tile([8, 2], F32, tag="rqd")             # rq relocated to base 0
    A1 = pers.tile([8, 16], F32, tag="A1")
    A2 = pers.tile([8, 16], F32, tag="A2")
    expA = pers.tile([8, 16], F32, tag="expA")
    eden = pers.tile([8, 2], F32, tag="eden")
    rd8 = pers.tile([8, 2], F32, tag="rd8")
    rhs24T = pers.tile([8, 48], F32, tag="rhs24T")      # 24s col-block
    rhs24 = [pers.tile([24, 8], BF16, tag=f"rhs24_{s}", name=f"rhs24_{s}") for s in range(2)]
    WT = pers.tile([128, 16], F32, tag="WT") if apply_wb else None
    BT = pers.tile([128, 16], F32, tag="BT") if apply_wb else None

    def mu_col(b, s):
        return MT[:, 74 * (b % 4) + 32 * s + b // 4: 74 * (b % 4) + 32 * s + b // 4 + 1]

    def rs_col(b, s):
        o = 74 * (b % 4) + 37 + 32 * s + b // 4
        return MT[:, o:o + 1]

    def fg_col(b, s):
        return MKT[:, 6 * b + s: 6 * b + s + 1]

    def bb_col(b, s):
        return MKT[:, 6 * b + 2 + s: 6 * b + 3 + s]

    def b_col(b, s):
        return MKT[:, 6 * b + 4 + s: 6 * b + 5 + s]

    def fnt(b, s):
        return FnT[:, 16 * b + 8 * s: 16 * b + 8 * s + 8]

    # =============== Phase A ===============
    with tc.tile_pool(name="psA", bufs=2, space="PSUM") as psA, \
         tc.tile_pool(name="psT", bufs=3, space="PSUM") as psT:

        nc.sync.dma_start(eye[:], ins["eye"])
        nc.sync.dma_start(eyeb[:], ins["eyeb"])
        nc.sync.dma_start(selsum[:], ins["selsum"])
        nc.sync.dma_start(wb16[:], ins["wb16"])
        nc.sync.dma_start(ones16[:], ins["ones16"])
        nc.vector.memset(consts[:, 0:1], 1e-5)
        for s in range(2):
            nc.gpsimd.memset(trTav[s][:], 0.0)
        for off, w in CHUNKS:
            nc.sync.dma_start(F128[0:64, off:off + w], ins["Fb0"][:, off:off + w])
            nc.sync.dma_start(F128[64:128, off:off + w], ins["Fb1"][:, off:off + w])
        nc.sync.dma_start(F16[0:8, :], ins["F8_0"])
        nc.sync.dma_start(F16[8:16, :], ins["F8_1"])

        # ---- masks (sobel in image space, batched samples in free dim) ----
        sv = sobm[:].rearrange("p (i s c) -> p i s c", s=2, c=50)

        def slot(i, r=(1, 49)):
            return sv[:, i, :, r[0]:r[1]]

        nc.gpsimd.memset(sobm[:, 0:200], 0.0)
        nc.sync.dma_start(slot(S_P50)[:, 0, :], ins["P0"])
        nc.sync.dma_start(slot(S_P50)[:, 1, :], ins["P1"])
        nc.scalar.activation(slot(S_PM), slot(S_P50), AF.Sigmoid)
        # exp table warmup (avoid mid-phase-B table load)
        nc.scalar.activation(sq[0:1, 0:1], consts[0:1, 0:1], AF.Exp)
        Pm0 = sv[:, S_PM]
        nc.vector.tensor_tensor(slot(S_A1), Pm0[:, :, 0:48], Pm0[:, :, 2:50], OP.subtract)
        nc.vector.tensor_tensor(slot(S_TMP), Pm0[:, :, 0:48], Pm0[:, :, 2:50], OP.add)
        nc.vector.scalar_tensor_tensor(slot(S_B1), Pm0[:, :, 1:49], 2.0, slot(S_TMP),
                                       OP.mult, OP.add)
        nc.gpsimd.memset(sobm[:, 100 * S_A1P:100 * S_A1P + 200], 0.0)
        for s in range(2):
            pt1 = psT.tile([128, 512], F32, tag="psT", name=f"pt1_{s}")
            nc.tensor.transpose(pt1[0:48, 0:48], slot(S_A1)[:, s, :], eye[0:48, 0:48])
            nc.vector.tensor_copy(slot(S_A1P)[:, s, :], pt1[0:48, 0:48])
            pt2 = psT.tile([128, 512], F32, tag="psT", name=f"pt2_{s}")
            nc.tensor.transpose(pt2[0:48, 0:48], slot(S_B1)[:, s, :], eye[0:48, 0:48])
            nc.vector.tensor_copy(slot(S_B1P)[:, s, :], pt2[0:48, 0:48])
        A1p = sv[:, S_A1P]
        B1p = sv[:, S_B1P]
        nc.vector.tensor_tensor(slot(S_TCOL), A1p[:, :, 0:48], A1p[:, :, 2:50], OP.add)
        nc.vector.scalar_tensor_tensor(slot(S_GXT), A1p[:, :, 1:49], 2.0, slot(S_TCOL),
                                       OP.mult, OP.add)
        nc.vector.tensor_tensor(slot(S_GYT), B1p[:, :, 0:48], B1p[:, :, 2:50], OP.subtract)
        nc.vector.tensor_tensor(slot(S_M1), slot(S_GXT), slot(S_GXT), OP.mult)
        nc.vector.tensor_tensor(slot(S_M2), slot(S_GYT), slot(S_GYT), OP.mult)
        nc.vector.tensor_tensor(slot(S_STT), slot(S_M1), slot(S_M2), OP.add)
        nc.vector.tensor_scalar(slot(S_BTM), slot(S_STT), 0.0, None, OP.is_gt)
        for s in range(2):
            pt3 = psT.tile([128, 512], F32, tag="psT", name=f"pt3_{s}")
            nc.tensor.transpose(pt3[0:48, 0:48], slot(S_BTM)[:, s, :], eye[0:48, 0:48])
            nc.vector.tensor_copy(slot(S_BHW)[:, s, :], pt3[0:48, 0:48])
        nc.vector.tensor_scalar(slot(S_FG), slot(S_P50), 0.0, None, OP.is_gt)
        nc.vector.tensor_scalar(slot(S_BG), slot(S_P50), 0.0, None, OP.is_lt)
        nc.vector.scalar_tensor_tensor(slot(S_BB), slot(S_BG), 1.0, slot(S_BHW),
                                       OP.mult, OP.max)
        # flatten masks to rows, then per-block transposes -> MKT
        for m, si in ((0, S_FG), (2, S_BB), (4, S_BHW)):
            for s in range(2):
                nc.sync.dma_start(mrows[m + s:m + s + 1, :], slot(si)[:, s, :])
        for b in range(NB):
            pm = psT.tile([128, 512], F32, tag="psT", name=f"pm{b}")
            nc.tensor.transpose(pm[0:128, 0:6], mrows[:, 128 * b:128 * b + 128],
                                eye[0:6, 0:6])
            nc.vector.tensor_copy(MKT[:, 6 * b:6 * b + 6], pm[0:128, 0:6])

        # ---- LayerNorm stats (channel-major PE reduction) ----
        for off, w in CHUNKS:
            nc.vector.tensor_tensor(Fsq[:, off:off + w], F128[:, off:off + w],
                                    F128[:, off:off + w], OP.mult)
        psumsA = psA.tile([128, 512], F32, tag="psA", name="psumsA")
        psumsB = psA.tile([128, 512], F32, tag="psA", name="psumsB")
        for c, (off, w) in enumerate(CHUNKS):
            nc.tensor.matmul(psumsA[0:37, 0:w], selsum[:, 37 * c:37 * c + 37],
                             F128[:, off:off + w], start=(c == 0), stop=(c == 4))
            nc.tensor.matmul(psumsB[0:37, 0:w], selsum[:, 37 * c:37 * c + 37],
                             Fsq[:, off:off + w], start=(c == 0), stop=(c == 4))
        s2 = stm[:, 0:512]
        varT = stm[:, 512:1024]
        sd = stm[:, 1024:1536]
        rstd = stm[:, 1536:2048]
        mu = stm[:, 2048:2560]
        nc.scalar.activation(s2, psumsA[0:37, :], AF.Square, scale=0.125)
        nc.vector.scalar_tensor_tensor(varT, psumsB[0:37, :], 1.0, s2, OP.mult, OP.subtract)
        nc.scalar.activation(sd, varT, AF.Sqrt, bias=consts[0:37, 0:1], scale=1.0 / 64.0)
        nc.vector.reciprocal(rstd, sd)
        nc.vector.tensor_scalar(mu, psumsA[0:37, :], 1.0 / 64.0, None, OP.mult)
        # stats -> pixel-major MT via transposes
        for j in range(4):
            for t, src in ((0, mu), (1, rstd)):
                pst = psT.tile([128, 512], F32, tag="psT", name=f"pst{j}_{t}")
                nc.tensor.transpose(pst[0:128, 0:37], src[:, 128 * j:128 * j + 128],
                                    eye[0:37, 0:37])
                nc.vector.tensor_copy(MT[:, 74 * j + 37 * t:74 * j + 37 * t + 37],
                                      pst[0:128, 0:37])

        if apply_wb:
            # broadcast per-channel w/b across partitions via ones-matmul
            pw = psT.tile([128, 512], F32, tag="psT", name="pw")
            nc.tensor.transpose(pw[0:2, 0:16], wb16[:], eye[0:16, 0:16])
            wbrow = sm.tile([2, 16], F32, tag="wbrow")
            nc.vector.tensor_copy(wbrow[:], pw[0:2, 0:16])
            onesr = sm.tile([1, 128], F32, tag="onesr")
            nc.vector.memset(onesr[:], 1.0)
            pw2 = psT.tile([128, 512], F32, tag="psT", name="pw2")
            nc.tensor.matmul(pw2[0:128, 0:16], onesr[:], wbrow[0:1, :], start=True, stop=True)
            nc.vector.tensor_copy(WT[:], pw2[0:128, 0:16])
            pw3 = psT.tile([128, 512], F32, tag="psT", name="pw3")
            nc.tensor.matmul(pw3[0:128, 0:16], onesr[:], wbrow[1:2, :], start=True, stop=True)
            nc.vector.tensor_copy(BT[:], pw3[0:128, 0:16])

        # ---- per-block: Fn_T, norms, q_T ----
        for b in range(NB):
            pF = psT.tile([128, 512], F32, tag="psT", name=f"pF{b}")
            nc.tensor.transpose(pF[0:128, 0:16], F16[:, 128 * b:128 * b + 128],
                                eye[0:16, 0:16])
            for s in range(2):
                nc.vector.tensor_scalar(fnt(b, s), pF[0:128, 8 * s:8 * s + 8],
                                        mu_col(b, s), rs_col(b, s), OP.subtract, OP.mult)
            if apply_wb:
                nc.gpsimd.tensor_tensor(FnT[:, 16 * b:16 * b + 16],
                                        FnT[:, 16 * b:16 * b + 16], WT[:], OP.mult)
                nc.gpsimd.tensor_tensor(FnT[:, 16 * b:16 * b + 16],
                                        FnT[:, 16 * b:16 * b + 16], BT[:], OP.add)
            for s in range(2):
                nc.vector.scalar_tensor_tensor(sq[:, 8 * s:8 * s + 8], fnt(b, s), 1.0,
                                               fnt(b, s), OP.mult, OP.mult,
                                               accum_out=NRM[:, 2 * b + s:2 * b + s + 1])
        nc.scalar.activation(RQB[:, 0:36], NRM[:], AF.Sqrt)
        nc.vector.tensor_scalar(RQB[:, 0:36], RQB[:, 0:36], 1e-12, None, OP.max)
        nc.vector.reciprocal(RQB[:, 36:72], RQB[:, 0:36])
        for b in range(NB):
            for s in range(2):
                nc.vector.tensor_scalar(qT[:, 16 * b + 8 * s:16 * b + 8 * s + 8],
                                        fnt(b, s), RQB[:, 36 + 2 * b + s:37 + 2 * b + s],
                                        None, OP.mult)
            pQ = psT.tile([128, 512], F32, tag="psT", name=f"pQ{b}")
            nc.tensor.transpose(pQ[0:16, 0:128], qT[:, 16 * b:16 * b + 16], eye[:, :])
            nc.scalar.activation(qcm16[:, 128 * b:128 * b + 128],
                                 pQ[0:16, 0:128], AF.Copy)
        nc.sync.dma_start(qcm1[:], qcm16[8:16, :])

        # ---- masked features: AV stationary (fp8) + channel-path tiles ----
        for b in range(NB):
            for s in range(2):
                nc.vector.tensor_scalar(trTav[s][:, 16 * b:16 * b + 8], fnt(b, s),
                                        fg_col(b, s), None, OP.mult)
                nc.gpsimd.tensor_scalar(trTav[s][:, 16 * b + 8:16 * b + 9], fg_col(b, s),
                                        1.0, None, OP.mult)
                base = 48 * b + 24 * s
                nc.gpsimd.tensor_scalar(bfg[:, base:base + 8], fnt(b, s), fg_col(b, s),
                                        None, OP.mult)
                nc.gpsimd.tensor_scalar(bfg[:, base + 8:base + 16], fnt(b, s),
                                        bb_col(b, s), None, OP.mult)
                nc.gpsimd.tensor_scalar(bfg[:, base + 16:base + 24], fnt(b, s), 1.0,
                                        None, OP.mult)
        for b in range(NB):
            for s in range(2):
                pC = psT.tile([128, 512], F32, tag="psT", name=f"pC{b}_{s}")
                nc.tensor.transpose(pC[0:24, 0:128],
                                    bfg[:, 48 * b + 24 * s:48 * b + 24 * s + 24],
                                    eye[:, :])
                nc.vector.tensor_copy(CM[s][:, 128 * b:128 * b + 128],
                                      pC[0:24, 0:128])
        # w1 = Fn + b*(q - Fn)   (final out = w1 + Fn + b*spat + Fch + qc)
        for b in range(NB):
            nc.gpsimd.tensor_tensor(w1[:, 16 * b:16 * b + 16], qT[:, 16 * b:16 * b + 16],
                                    FnT[:, 16 * b:16 * b + 16], OP.subtract)
            for s in range(2):
                nc.vector.scalar_tensor_tensor(w1[:, 16 * b + 8 * s:16 * b + 8 * s + 8],
                                               w1[:, 16 * b + 8 * s:16 * b + 8 * s + 8],
                                               b_col(b, s), fnt(b, s), OP.mult, OP.add)

    # =============== Phase B + channel path + finals ===============
    Sv = Sall[:].rearrange("p (t b x) -> p t b x", t=2, b=NB)
    tv = [trTav[s][:].rearrange("p (pb i c) -> p pb i c", i=2, c=16) for s in range(2)]

    with tc.tile_pool(name="psL", bufs=2, space="PSUM") as psL, \
         tc.tile_pool(name="psO", bufs=1, space="PSUM") as psO, \
         tc.tile_pool(name="psM", bufs=1, space="PSUM") as psMp, \
         tc.tile_pool(name="sS", bufs=2) as sS:
        psM = psMp.tile([128, 512], F32, tag="psM")
        kslot = [0]

        def phase_b(s):
            qsrc = qcm16[0:8, :] if s == 0 else qcm1[:]
            for jc, (joff, jw) in enumerate(CHUNKS):
                Sb = Sv[:, jc % 2]
                psOt = psO.tile([128, 512], F32, tag="psO", name=f"psO{s}_{jc}")

                def logits(g):
                    Lg = psL.tile([128, GRP * 512], F32, tag="L", name=f"L{s}_{jc}_{g}")
                    Lv = Lg[:].rearrange("p (i x) -> p i x", i=GRP)
                    for i in range(GRP):
                        b = GRP * g + i
                        nc.tensor.matmul(Lv[:, i, 0:jw], qsrc[:, 128 * b:128 * b + 128],
                                         qsrc[:, joff:joff + jw],
                                         start=True, stop=True)
                    nc.scalar.activation(Sb[:, GRP * g:GRP * g + GRP, 0:jw],
                                         Lv[:, :, 0:jw], AF.Exp)

                def av(pb):
                    nc.tensor.matmul(psOt[0:16, 0:jw], tv[s][:, pb], Sb[:, 2 * pb:2 * pb + 2, 0:jw],
                                     start=(pb == 0), stop=(pb == 8), perf_mode=DR)

                sched = {1: [0], 2: [1, 2], 3: [3], 4: [4, 5], 5: [6]}
                logits(0)
                for g in range(1, 6):
                    logits(g)
                    for pb in sched[g]:
                        av(pb)
                av(7)
                av(8)
                # transposed epilogue: spat = num/den, pixel-major
                avs = sS.tile([9, 512], F32, tag="avs", name=f"avs{s}_{jc}")
                nc.vector.tensor_copy(avs[:, 0:jw], psOt[0:9, 0:jw])
                for j in range(jw // 128):
                    b = 4 * jc + j
                    k = kslot[0] % 8
                    kslot[0] += 1
                    pslot = psM[0:128, 9 * k:9 * k + 9]
                    nc.tensor.transpose(pslot, avs[:, 128 * j:128 * j + 128], eye[0:9, 0:9])
                    nc.vector.reciprocal(rc[:, k:k + 1], pslot[:, 8:9])
                    nc.vector.tensor_scalar(spatT[:, 16 * b + 8 * s:16 * b + 8 * s + 8],
                                            pslot[:, 0:8], rc[:, k:k + 1], None, OP.mult)

        def channel_path():
            # per-sample Gram accumulation: psum16 = [fg|bb].T @ [fg|bb] over pixels
            ps16 = [psM[0:16, 72 + 16 * s:72 + 16 * s + 16] for s in range(2)]
            for s in range(2):
                for b in range(NB):
                    base = 48 * b + 24 * s
                    nc.tensor.matmul(ps16[s], bfg[:, base:base + 16], bfg[:, base:base + 16],
                                     start=(b == 0), stop=(b == NB - 1))
            for s in range(2):
                nc.vector.tensor_tensor(msk[:, 16 * s:16 * s + 16], ps16[s],
                                        eye[0:16, 0:16], OP.mult)
                pd = psM[0:16, 104 + 2 * s:104 + 2 * s + 1]
                nc.tensor.matmul(pd, msk[:, 16 * s:16 * s + 16], ones16[:],
                                 start=True, stop=True)
                nc.scalar.activation(r16f[:, s:s + 1], pd, AF.Sqrt)
            nc.vector.tensor_scalar(r16f[:, 0:2], r16f[:, 0:2], 1e-12, None, OP.max)
            nc.vector.reciprocal(r16f[:, 2:4], r16f[:, 0:2])
            # rq (rows 8:16 of r16f) relocated to base 0
            nc.sync.dma_start(rqd[:], r16f[8:16, 2:4])
            for s in range(2):
                # Gram is symmetric: G^T[k, q] = Gram[0:8, 8:16] (fg rows, bb cols)
                nc.vector.tensor_scalar(A1[:, 8 * s:8 * s + 8],
                                        ps16[s][0:8, 8:16],
                                        r16f[0:8, 2 + s:3 + s], None, OP.mult)
                pA = psM[0:8, 140 + 8 * s:140 + 8 * s + 8]
                nc.tensor.transpose(pA, A1[:, 8 * s:8 * s + 8], eye[0:8, 0:8])
                nc.vector.tensor_scalar(A2[:, 8 * s:8 * s + 8], pA, rqd[:, s:s + 1],
                                        None, OP.mult)
                nc.scalar.activation(expA[:, 8 * s:8 * s + 8], A2[:, 8 * s:8 * s + 8],
                                     AF.Exp, accum_out=eden[:, s:s + 1])
                nc.vector.reciprocal(rd8[:, s:s + 1], eden[:, s:s + 1])
                # rhs24T[q, :] = [attn_c[q, k], diag(rq)[q, k], I8[q, d]] -> transpose
                base = 24 * s
                nc.vector.tensor_scalar(rhs24T[:, base:base + 8], expA[:, 8 * s:8 * s + 8],
                                        rd8[:, s:s + 1], None, OP.mult)
                nc.vector.tensor_scalar(rhs24T[:, base + 8:base + 16], eye[0:8, 0:8],
                                        rqd[:, s:s + 1], None, OP.mult)
                nc.vector.tensor_copy(rhs24T[:, base + 16:base + 24], eye[0:8, 0:8])
                pR = psM[0:24, 156 + 8 * s:156 + 8 * s + 8]
                nc.tensor.transpose(pR, rhs24T[:, base:base + 24], eye[0:8, 0:8])
                nc.vector.tensor_copy(rhs24[s][:], pR)

        def finals(s):
            for b in range(NB):
                fslot = psM[0:128, 172 + 8 * (b % 2):172 + 8 * (b % 2) + 8]
                nc.tensor.matmul(fslot, CM[s][:, 128 * b:128 * b + 128], rhs24[s][:],
                                 start=True, stop=True)
                o = OUTT[:, 16 * b + 8 * s:16 * b + 8 * s + 8]
                nc.vector.scalar_tensor_tensor(o, spatT[:, 16 * b + 8 * s:16 * b + 8 * s + 8],
                                               b_col(b, s),
                                               w1[:, 16 * b + 8 * s:16 * b + 8 * s + 8],
                                               OP.mult, OP.add)
                nc.vector.tensor_tensor(o, o, fslot, OP.add)
                tslot = psM[0:8, 192 + 128 * (b % 2):192 + 128 * (b % 2) + 128]
                nc.tensor.transpose(tslot, o, eye[:, :])
                nc.vector.tensor_copy(fin[s][:, 128 * b:128 * b + 128], tslot)
            nc.sync.dma_start(outs[s][:], fin[s][:])

        phase_b(0)
        channel_path()
        finals(0)
        phase_b(1)
        finals(1)


_PROGRAMS = {}


def _program(apply_wb=False):
    if apply_wb not in _PROGRAMS:
        _PROGRAMS[apply_wb] = build_program(apply_wb)
    return _PROGRAMS[apply_wb]


def kernel(F, P, norm_weight, norm_bias):
    from concourse.bass_utils import run_bass_kernel_spmd
    w = np.asarray(norm_weight, np.float32)
    b = np.asarray(norm_bias, np.float32)
    apply_wb = not (np.all(w == 1.0) and np.all(b == 0.0))
    nc = _program(apply_wb)
    maps = make_inmaps(F, P, norm_weight, norm_bias)
    res = run_bass_kernel_spmd(nc, maps, core_ids=list(range(8)), trace=False)
    return assemble(res.results)


# revision 4
# speedup vs baseline: 1.2250x; 1.0239x over previous
"""Trainium2 Bass kernel for nn_BRC_62715112457019 (sparse_attention), v2.

Head-parallel across 8 cores (core c = head c, both samples). Pixel-major
phase A (per-pixel stats/masks live on partitions -> no broadcast DMAs, tiny
128-wide DVE ops), fp8 DoubleRow phase B (QK^T and AV at 2 fp8 MACs/cycle),
per-qchunk transposed epilogue (no row-broadcasts), overlapped channel-attn
path and output writeback.

Pixel blocking: block b in [0,18) covers pixels [128b, 128b+128). Pixel-major
tiles are [128, 18*K] with column group b. Channel-major tensors ([8|16, HW])
are produced/consumed via PE transposes per block.
"""
import sys
for _p in ('/opt/trn_rl_repo', '/opt/pypackages'):
    if _p not in sys.path:
        sys.path.insert(0, _p)
import numpy as np
import ml_dtypes
from contextlib import ExitStack

import concourse.bass as bass
import concourse.bacc as bacc
import concourse.tile as tile
from concourse import mybir

dt = mybir.dt
F32 = dt.float32
BF16 = dt.bfloat16
FP8 = dt.float8e4
AF = mybir.ActivationFunctionType
OP = mybir.AluOpType
DR = mybir.MatmulPerfMode.DoubleRow

HW = 2304
NB = 18                       # 128-pixel blocks
CHUNKS = [(0, 512), (512, 512), (1024, 512), (1536, 512), (2048, 256)]
GRP = 3                       # logit blocks per exp group
BF = ml_dtypes.bfloat16
F8 = ml_dtypes.float8_e4m3fn


def host_constants(w8, b8):
    eye = np.eye(128, dtype=np.float32)
    eyeb = np.eye(128, dtype=BF)
    selsum = np.zeros((128, 5 * 37), BF)
    for c in range(5):
        selsum[0:64, 37 * c + c] = 1.0
        selsum[64:128, 37 * c + 32 + c] = 1.0
    wb16 = np.zeros((16, 2), np.float32)
    wb16[0:8, 0] = w8
    wb16[8:16, 0] = w8
    wb16[0:8, 1] = b8
    wb16[8:16, 1] = b8
    ones16 = np.ones((16, 1), np.float32)
    return {"eye": eye, "eyeb": eyeb, "selsum": selsum, "wb16": wb16,
            "ones16": ones16}


def make_inmaps(F, P, norm_weight, norm_bias):
    F = np.asarray(F, np.float32).reshape(2, 64, HW)
    P = np.asarray(P, np.float32).reshape(2, 48, 48)
    w = np.asarray(norm_weight, np.float32)
    b = np.asarray(norm_bias, np.float32)
    maps = []
    for c in range(8):
        m = host_constants(w[8 * c:8 * c + 8], b[8 * c:8 * c + 8])
        for n in range(2):
            m[f"Fb{n}"] = np.ascontiguousarray(F[n].astype(BF))
            m[f"F8_{n}"] = np.ascontiguousarray(F[n, 8 * c:8 * c + 8])
            m[f"P{n}"] = np.ascontiguousarray(P[n])
        maps.append(m)
    return maps


def assemble(results):
    out = np.empty((2, 64, 48, 48), np.float32)
    for c in range(8):
        for n in range(2):
            out[n, 8 * c:8 * c + 8] = results[c][f"out{n}"].reshape(8, 48, 48)
    return out


def build_program(apply_wb):
    nc = bacc.Bacc("TRN2", target_bir_lowering=False, debug=False)
    ins = {}
    for n in range(2):
        ins[f"Fb{n}"] = nc.dram_tensor(f"Fb{n}", [64, HW], BF16, kind="ExternalInput").ap()
        ins[f"F8_{n}"] = nc.dram_tensor(f"F8_{n}", [8, HW], F32, kind="ExternalInput").ap()
        ins[f"P{n}"] = nc.dram_tensor(f"P{n}", [48, 48], F32, kind="ExternalInput").ap()
    ins["eye"] = nc.dram_tensor("eye", [128, 128], F32, kind="ExternalInput").ap()
    ins["eyeb"] = nc.dram_tensor("eyeb", [128, 128], BF16, kind="ExternalInput").ap()
    ins["selsum"] = nc.dram_tensor("selsum", [128, 185], BF16, kind="ExternalInput").ap()
    ins["wb16"] = nc.dram_tensor("wb16", [16, 2], F32, kind="ExternalInput").ap()
    ins["ones16"] = nc.dram_tensor("ones16", [16, 1], F32, kind="ExternalInput").ap()
    outs = [nc.dram_tensor(f"out{n}", [8, HW], F32, kind="ExternalOutput").ap() for n in range(2)]

    with tile.TileContext(nc) as tc:
        with ExitStack() as ctx:
            _body(ctx, tc, nc, ins, outs, apply_wb)
    nc.compile()
    return nc


# sobel slot indices (pairs of 50 cols: sample0|sample1, rows 0:48)
S_P50, S_PM, S_A1, S_TMP, S_B1, S_A1P, S_B1P, S_TCOL, S_GXT, S_GYT, S_M1, S_M2, \
    S_STT, S_BTM, S_BHW, S_FG, S_BG, S_BB = range(18)


def _body(ctx, tc, nc, ins, outs, apply_wb):
    pers = ctx.enter_context(tc.tile_pool(name="pers", bufs=1))
    sm = ctx.enter_context(tc.tile_pool(name="sm", bufs=1))

    # ---- persistent tiles ----
    eye = pers.tile([128, 128], F32, tag="eye")
    eyeb = pers.tile([128, 128], BF16, tag="eyeb")
    selsum = pers.tile([128, 185], BF16, tag="selsum")
    wb16 = pers.tile([16, 2], F32, tag="wb16")
    ones16 = pers.tile([16, 1], F32, tag="ones16")
    consts = pers.tile([128, 2], F32, tag="consts")     # col0 = eps
    F128 = pers.tile([128, HW], BF16, tag="F128")
    Fsq = pers.tile([128, HW], BF16, tag="Fsq")
    F16 = pers.tile([16, HW], F32, tag="F16")
    FnT = pers.tile([128, 288], F32, tag="FnT")         # 16b+8s+d
    qT = pers.tile([128, 288], F32, tag="qT")
    NRM = pers.tile([128, 36], F32, tag="NRM")          # 2b+s
    RQB = pers.tile([128, 72], F32, tag="RQB")          # [0:36] sqrt, [36:72] recip
    MT = pers.tile([128, 296], F32, tag="MT")           # 74j+37t+32s+c
    MKT = pers.tile([128, 108], F32, tag="MKT")         # 6b+{fg0,fg1,bb0,bb1,b0,b1}
    mrows = pers.tile([6, HW], F32, tag="mrows")
    qcm16 = pers.tile([16, HW], BF16, tag="qcm16")
    qcm1 = pers.tile([8, HW], BF16, tag="qcm1")
    trTav = [pers.tile([128, 288], FP8, tag=f"trTav{s}", name=f"trTav{s}") for s in range(2)]
    bfg = pers.tile([128, 864], F32, tag="bfg")        # 48b+24s+{fg8,bb8,Fn8}
    CM = [pers.tile([24, HW], BF16, tag=f"CM{s}", name=f"CM{s}") for s in range(2)]
    Sall = pers.tile([128, 2 * NB * 512], FP8, tag="Sall")
    w1 = pers.tile([128, 288], F32, tag="w1")
    spatT = pers.tile([128, 288], F32, tag="spatT")
    OUTT = pers.tile([128, 288], F32, tag="OUTT")
    fin = [pers.tile([8, HW], F32, tag=f"fin{s}", name=f"fin{s}") for s in range(2)]
    rc = pers.tile([128, 8], F32, tag="rc")             # epilogue denominators
    sobm = pers.tile([48, 100 * 18], F32, tag="sobm")
    stm = pers.tile([37, 2560], F32, tag="stm")
    sq = pers.tile([128, 16], F32, tag="sq")
    # channel path smalls
    msk = pers.tile([16, 32], F32, tag="msk")
    r16f = pers.tile([16, 4], F32, tag="r16f")          # [0:2] sqrt, [2:4]=1/max(sqrt,..); col s
    rqd = pers.tile([8, 2], F32, tag="rqd")             # rq relocated to base 0
    A1 = pers.tile([8, 16], F32, tag="A1")
    A2 = pers.tile([8, 16], F32, tag="A2")
    expA = pers.tile([8, 16], F32, tag="expA")
    eden = pers.tile([8, 2], F32, tag="eden")
    rd8 = pers.tile([8, 2], F32, tag="rd8")
    rhs24T = pers.tile([8, 48], F32, tag="rhs24T")      # 24s col-block
    rhs24 = [pers.tile([24, 8], BF16, tag=f"rhs24_{s}", name=f"rhs24_{s}") for s in range(2)]
    WT = pers.tile([128, 16], F32, tag="WT") if apply_wb else None
    BT = pers.tile([128, 16], F32, tag="BT") if apply_wb else None

    def mu_col(b, s):
        return MT[:, 74 * (b % 4) + 32 * s + b // 4: 74 * (b % 4) + 32 * s + b // 4 + 1]

    def rs_col(b, s):
        o = 74 * (b % 4) + 37 + 32 * s + b // 4
        return MT[:, o:o + 1]

    def fg_col(b, s):
        return MKT[:, 6 * b + s: 6 * b + s + 1]

    def bb_col(b, s):
        return MKT[:, 6 * b + 2 + s: 6 * b + 3 + s]

    def b_col(b, s):
        return MKT[:, 6 * b + 4 + s: 6 * b + 5 + s]

    def fnt(b, s):
        return FnT[:, 16 * b + 8 * s: 16 * b + 8 * s + 8]

    # =============== Phase A ===============
    with tc.tile_pool(name="psA", bufs=2, space="PSUM") as psA, \
         tc.tile_pool(name="psT", bufs=3, space="PSUM") as psT:

        nc.sync.dma_start(eye[:], ins["eye"])
        nc.sync.dma_start(eyeb[:], ins["eyeb"])
        nc.sync.dma_start(selsum[:], ins["selsum"])
        nc.sync.dma_start(wb16[:], ins["wb16"])
        nc.sync.dma_start(ones16[:], ins["ones16"])
        nc.vector.memset(consts[:, 0:1], 1e-5)
        for s in range(2):
            nc.vector.memset(trTav[s][:], 0.0)
        for off, w in CHUNKS:
            nc.sync.dma_start(F128[0:64, off:off + w], ins["Fb0"][:, off:off + w])
            nc.sync.dma_start(F128[64:128, off:off + w], ins["Fb1"][:, off:off + w])
        nc.sync.dma_start(F16[0:8, :], ins["F8_0"])
        nc.sync.dma_start(F16[8:16, :], ins["F8_1"])

        # ---- masks (sobel in image space, batched samples in free dim) ----
        sv = sobm[:].rearrange("p (i s c) -> p i s c", s=2, c=50)

        def slot(i, r=(1, 49)):
            return sv[:, i, :, r[0]:r[1]]

        nc.gpsimd.memset(sobm[:, 0:200], 0.0)
        nc.sync.dma_start(slot(S_P50)[:, 0, :], ins["P0"])
        nc.sync.dma_start(slot(S_P50)[:, 1, :], ins["P1"])
        nc.scalar.activation(slot(S_PM), slot(S_P50), AF.Sigmoid)
        # exp table warmup (avoid mid-phase-B table load)
        nc.scalar.activation(sq[0:1, 0:1], consts[0:1, 0:1], AF.Exp)
        Pm0 = sv[:, S_PM]
        nc.vector.tensor_tensor(slot(S_A1), Pm0[:, :, 0:48], Pm0[:, :, 2:50], OP.subtract)
        nc.vector.tensor_tensor(slot(S_TMP), Pm0[:, :, 0:48], Pm0[:, :, 2:50], OP.add)
        nc.vector.scalar_tensor_tensor(slot(S_B1), Pm0[:, :, 1:49], 2.0, slot(S_TMP),
                                       OP.mult, OP.add)
        nc.gpsimd.memset(sobm[:, 100 * S_A1P:100 * S_A1P + 200], 0.0)
        for s in range(2):
            pt1 = psT.tile([128, 512], F32, tag="psT", name=f"pt1_{s}")
            nc.tensor.transpose(pt1[0:48, 0:48], slot(S_A1)[:, s, :], eye[0:48, 0:48])
            nc.vector.tensor_copy(slot(S_A1P)[:, s, :], pt1[0:48, 0:48])
            pt2 = psT.tile([128, 512], F32, tag="psT", name=f"pt2_{s}")
            nc.tensor.transpose(pt2[0:48, 0:48], slot(S_B1)[:, s, :], eye[0:48, 0:48])
            nc.vector.tensor_copy(slot(S_B1P)[:, s, :], pt2[0:48, 0:48])
        A1p = sv[:, S_A1P]
        B1p = sv[:, S_B1P]
        nc.vector.tensor_tensor(slot(S_TCOL), A1p[:, :, 0:48], A1p[:, :, 2:50], OP.add)
        nc.vector.scalar_tensor_tensor(slot(S_GXT), A1p[:, :, 1:49], 2.0, slot(S_TCOL),
                                       OP.mult, OP.add)
        nc.vector.tensor_tensor(slot(S_GYT), B1p[:, :, 0:48], B1p[:, :, 2:50], OP.subtract)
        nc.vector.tensor_tensor(slot(S_M1), slot(S_GXT), slot(S_GXT), OP.mult)
        nc.vector.tensor_tensor(slot(S_M2), slot(S_GYT), slot(S_GYT), OP.mult)
        nc.vector.tensor_tensor(slot(S_STT), slot(S_M1), slot(S_M2), OP.add)
        nc.vector.tensor_scalar(slot(S_BTM), slot(S_STT), 0.0, None, OP.is_gt)
        for s in range(2):
            pt3 = psT.tile([128, 512], F32, tag="psT", name=f"pt3_{s}")
            nc.tensor.transpose(pt3[0:48, 0:48], slot(S_BTM)[:, s, :], eye[0:48, 0:48])
            nc.vector.tensor_copy(slot(S_BHW)[:, s, :], pt3[0:48, 0:48])
        nc.vector.tensor_scalar(slot(S_FG), slot(S_P50), 0.0, None, OP.is_gt)
        nc.vector.tensor_scalar(slot(S_BG), slot(S_P50), 0.0, None, OP.is_lt)
        nc.vector.scalar_tensor_tensor(slot(S_BB), slot(S_BG), 1.0, slot(S_BHW),
                                       OP.mult, OP.max)
        # flatten masks to rows, then per-block transposes -> MKT
        for m, si in ((0, S_FG), (2, S_BB), (4, S_BHW)):
            for s in range(2):
                nc.sync.dma_start(mrows[m + s:m + s + 1, :], slot(si)[:, s, :])
        for b in range(NB):
            pm = psT.tile([128, 512], F32, tag="psT", name=f"pm{b}")
            nc.tensor.transpose(pm[0:128, 0:6], mrows[:, 128 * b:128 * b + 128],
                                eye[0:6, 0:6])
            nc.vector.tensor_copy(MKT[:, 6 * b:6 * b + 6], pm[0:128, 0:6])

        # ---- LayerNorm stats (channel-major PE reduction) ----
        for off, w in CHUNKS:
            nc.vector.tensor_tensor(Fsq[:, off:off + w], F128[:, off:off + w],
                                    F128[:, off:off + w], OP.mult)
        psumsA = psA.tile([128, 512], F32, tag="psA", name="psumsA")
        psumsB = psA.tile([128, 512], F32, tag="psA", name="psumsB")
        for c, (off, w) in enumerate(CHUNKS):
            nc.tensor.matmul(psumsA[0:37, 0:w], selsum[:, 37 * c:37 * c + 37],
                             F128[:, off:off + w], start=(c == 0), stop=(c == 4))
            nc.tensor.matmul(psumsB[0:37, 0:w], selsum[:, 37 * c:37 * c + 37],
                             Fsq[:, off:off + w], start=(c == 0), stop=(c == 4))
        s2 = stm[:, 0:512]
        varT = stm[:, 512:1024]
        sd = stm[:, 1024:1536]
        rstd = stm[:, 1536:2048]
        mu = stm[:, 2048:2560]
        nc.scalar.activation(s2, psumsA[0:37, :], AF.Square, scale=0.125)
        nc.vector.scalar_tensor_tensor(varT, psumsB[0:37, :], 1.0, s2, OP.mult, OP.subtract)
        nc.scalar.activation(sd, varT, AF.Sqrt, bias=consts[0:37, 0:1], scale=1.0 / 64.0)
        nc.vector.reciprocal(rstd, sd)
        nc.vector.tensor_scalar(mu, psumsA[0:37, :], 1.0 / 64.0, None, OP.mult)
        # stats -> pixel-major MT via transposes
        for j in range(4):
            for t, src in ((0, mu), (1, rstd)):
                pst = psT.tile([128, 512], F32, tag="psT", name=f"pst{j}_{t}")
                nc.tensor.transpose(pst[0:128, 0:37], src[:, 128 * j:128 * j + 128],
                                    eye[0:37, 0:37])
                nc.vector.tensor_copy(MT[:, 74 * j + 37 * t:74 * j + 37 * t + 37],
                                      pst[0:128, 0:37])

        if apply_wb:
            # broadcast per-channel w/b across partitions via ones-matmul
            pw = psT.tile([128, 512], F32, tag="psT", name="pw")
            nc.tensor.transpose(pw[0:2, 0:16], wb16[:], eye[0:16, 0:16])
            wbrow = sm.tile([2, 16], F32, tag="wbrow")
            nc.vector.tensor_copy(wbrow[:], pw[0:2, 0:16])
            onesr = sm.tile([1, 128], F32, tag="onesr")
            nc.vector.memset(onesr[:], 1.0)
            pw2 = psT.tile([128, 512], F32, tag="psT", name="pw2")
            nc.tensor.matmul(pw2[0:128, 0:16], onesr[:], wbrow[0:1, :], start=True, stop=True)
            nc.vector.tensor_copy(WT[:], pw2[0:128, 0:16])
            pw3 = psT.tile([128, 512], F32, tag="psT", name="pw3")
            nc.tensor.matmul(pw3[0:128, 0:16], onesr[:], wbrow[1:2, :], start=True, stop=True)
            nc.vector.tensor_copy(BT[:], pw3[0:128, 0:16])

        # ---- per-block: Fn_T, norms, q_T ----
        for b in range(NB):
            pF = psT.tile([128, 512], F32, tag="psT", name=f"pF{b}")
            nc.tensor.transpose(pF[0:128, 0:16], F16[:, 128 * b:128 * b + 128],
                                eye[0:16, 0:16])
            for s in range(2):
                nc.vector.tensor_scalar(fnt(b, s), pF[0:128, 8 * s:8 * s + 8],
                                        mu_col(b, s), rs_col(b, s), OP.subtract, OP.mult)
            if apply_wb:
                nc.vector.tensor_tensor(FnT[:, 16 * b:16 * b + 16],
                                        FnT[:, 16 * b:16 * b + 16], WT[:], OP.mult)
                nc.vector.tensor_tensor(FnT[:, 16 * b:16 * b + 16],
                                        FnT[:, 16 * b:16 * b + 16], BT[:], OP.add)
            for s in range(2):
                nc.vector.scalar_tensor_tensor(sq[:, 8 * s:8 * s + 8], fnt(b, s), 1.0,
                                               fnt(b, s), OP.mult, OP.mult,
                                               accum_out=NRM[:, 2 * b + s:2 * b + s + 1])
        nc.scalar.activation(RQB[:, 0:36], NRM[:], AF.Sqrt)
        nc.vector.tensor_scalar(RQB[:, 0:36], RQB[:, 0:36], 1e-12, None, OP.max)
        nc.vector.reciprocal(RQB[:, 36:72], RQB[:, 0:36])
        for b in range(NB):
            for s in range(2):
                nc.vector.tensor_scalar(qT[:, 16 * b + 8 * s:16 * b + 8 * s + 8],
                                        fnt(b, s), RQB[:, 36 + 2 * b + s:37 + 2 * b + s],
                                        None, OP.mult)
            pQ = psT.tile([128, 512], F32, tag="psT", name=f"pQ{b}")
            nc.tensor.transpose(pQ[0:16, 0:128], qT[:, 16 * b:16 * b + 16], eye[:, :])
            nc.scalar.activation(qcm16[:, 128 * b:128 * b + 128],
                                 pQ[0:16, 0:128], AF.Copy)
        nc.sync.dma_start(qcm1[:], qcm16[8:16, :])

        # ---- masked features: AV stationary (fp8) + channel-path tiles ----
        for b in range(NB):
            for s in range(2):
                nc.vector.tensor_scalar(trTav[s][:, 16 * b:16 * b + 8], fnt(b, s),
                                        fg_col(b, s), None, OP.mult)
                nc.vector.tensor_scalar(trTav[s][:, 16 * b + 8:16 * b + 9], fg_col(b, s),
                                        1.0, None, OP.mult)
                base = 48 * b + 24 * s
                nc.vector.tensor_scalar(bfg[:, base:base + 8], fnt(b, s), fg_col(b, s),
                                        None, OP.mult)
                nc.vector.tensor_scalar(bfg[:, base + 8:base + 16], fnt(b, s),
                                        bb_col(b, s), None, OP.mult)
                nc.vector.tensor_scalar(bfg[:, base + 16:base + 24], fnt(b, s), 1.0,
                                        None, OP.mult)
        for b in range(NB):
            for s in range(2):
                pC = psT.tile([128, 512], F32, tag="psT", name=f"pC{b}_{s}")
                nc.tensor.transpose(pC[0:24, 0:128],
                                    bfg[:, 48 * b + 24 * s:48 * b + 24 * s + 24],
                                    eye[:, :])
                nc.vector.tensor_copy(CM[s][:, 128 * b:128 * b + 128],
                                      pC[0:24, 0:128])
        # w1 = Fn + b*(q - Fn)   (final out = w1 + Fn + b*spat + Fch + qc)
        for b in range(NB):
            nc.vector.tensor_tensor(w1[:, 16 * b:16 * b + 16], qT[:, 16 * b:16 * b + 16],
                                    FnT[:, 16 * b:16 * b + 16], OP.subtract)
            for s in range(2):
                nc.vector.scalar_tensor_tensor(w1[:, 16 * b + 8 * s:16 * b + 8 * s + 8],
                                               w1[:, 16 * b + 8 * s:16 * b + 8 * s + 8],
                                               b_col(b, s), fnt(b, s), OP.mult, OP.add)

    # =============== Phase B + channel path + finals ===============
    Sv = Sall[:].rearrange("p (t b x) -> p t b x", t=2, b=NB)
    tv = [trTav[s][:].rearrange("p (pb i c) -> p pb i c", i=2, c=16) for s in range(2)]

    with tc.tile_pool(name="psL", bufs=2, space="PSUM") as psL, \
         tc.tile_pool(name="psO", bufs=1, space="PSUM") as psO, \
         tc.tile_pool(name="psM", bufs=1, space="PSUM") as psMp, \
         tc.tile_pool(name="sS", bufs=2) as sS:
        psM = psMp.tile([128, 512], F32, tag="psM")
        kslot = [0]

        def phase_b(s):
            qsrc = qcm16[0:8, :] if s == 0 else qcm1[:]
            for jc, (joff, jw) in enumerate(CHUNKS):
                Sb = Sv[:, jc % 2]
                psOt = psO.tile([128, 512], F32, tag="psO", name=f"psO{s}_{jc}")

                def logits(g):
                    Lg = psL.tile([128, GRP * 512], F32, tag="L", name=f"L{s}_{jc}_{g}")
                    Lv = Lg[:].rearrange("p (i x) -> p i x", i=GRP)
                    for i in range(GRP):
                        b = GRP * g + i
                        nc.tensor.matmul(Lv[:, i, 0:jw], qsrc[:, 128 * b:128 * b + 128],
                                         qsrc[:, joff:joff + jw],
                                         start=True, stop=True)
                    nc.scalar.activation(Sb[:, GRP * g:GRP * g + GRP, 0:jw],
                                         Lv[:, :, 0:jw], AF.Exp)

                def av(pb):
                    nc.tensor.matmul(psOt[0:16, 0:jw], tv[s][:, pb], Sb[:, 2 * pb:2 * pb + 2, 0:jw],
                                     start=(pb == 0), stop=(pb == 8), perf_mode=DR)

                sched = {1: [0], 2: [1, 2], 3: [3], 4: [4, 5], 5: [6]}
                logits(0)
                for g in range(1, 6):
                    logits(g)
                    for pb in sched[g]:
                        av(pb)
                av(7)
                av(8)
                # transposed epilogue: spat = num/den, pixel-major
                avs = sS.tile([9, 512], F32, tag="avs", name=f"avs{s}_{jc}")
                nc.vector.tensor_copy(avs[:, 0:jw], psOt[0:9, 0:jw])
                for j in range(jw // 128):
                    b = 4 * jc + j
                    k = kslot[0] % 8
                    kslot[0] += 1
                    pslot = psM[0:128, 9 * k:9 * k + 9]
                    nc.tensor.transpose(pslot, avs[:, 128 * j:128 * j + 128], eye[0:9, 0:9])
                    nc.vector.reciprocal(rc[:, k:k + 1], pslot[:, 8:9])
                    nc.vector.tensor_scalar(spatT[:, 16 * b + 8 * s:16 * b + 8 * s + 8],
                                            pslot[:, 0:8], rc[:, k:k + 1], None, OP.mult)

        def channel_path():
            # per-sample Gram accumulation: psum16 = [fg|bb].T @ [fg|bb] over pixels
            ps16 = [psM[0:16, 72 + 16 * s:72 + 16 * s + 16] for s in range(2)]
            for s in range(2):
                for b in range(NB):
                    base = 48 * b + 24 * s
                    nc.tensor.matmul(ps16[s], bfg[:, base:base + 16], bfg[:, base:base + 16],
                                     start=(b == 0), stop=(b == NB - 1))
            for s in range(2):
                nc.vector.tensor_tensor(msk[:, 16 * s:16 * s + 16], ps16[s],
                                        eye[0:16, 0:16], OP.mult)
                pd = psM[0:16, 104 + 2 * s:104 + 2 * s + 1]
                nc.tensor.matmul(pd, msk[:, 16 * s:16 * s + 16], ones16[:],
                                 start=True, stop=True)
                nc.scalar.activation(r16f[:, s:s + 1], pd, AF.Sqrt)
            nc.vector.tensor_scalar(r16f[:, 0:2], r16f[:, 0:2], 1e-12, None, OP.max)
            nc.vector.reciprocal(r16f[:, 2:4], r16f[:, 0:2])
            # rq (rows 8:16 of r16f) relocated to base 0
            nc.sync.dma_start(rqd[:], r16f[8:16, 2:4])
            for s in range(2):
                # Gram is symmetric: G^T[k, q] = Gram[0:8, 8:16] (fg rows, bb cols)
                nc.vector.tensor_scalar(A1[:, 8 * s:8 * s + 8],
                                        ps16[s][0:8, 8:16],
                                        r16f[0:8, 2 + s:3 + s], None, OP.mult)
                pA = psM[0:8, 140 + 8 * s:140 + 8 * s + 8]
                nc.tensor.transpose(pA, A1[:, 8 * s:8 * s + 8], eye[0:8, 0:8])
                nc.vector.tensor_scalar(A2[:, 8 * s:8 * s + 8], pA, rqd[:, s:s + 1],
                                        None, OP.mult)
                nc.scalar.activation(expA[:, 8 * s:8 * s + 8], A2[:, 8 * s:8 * s + 8],
                                     AF.Exp, accum_out=eden[:, s:s + 1])
                nc.vector.reciprocal(rd8[:, s:s + 1], eden[:, s:s + 1])
                # rhs24T[q, :] = [attn_c[q, k], diag(rq)[q, k], I8[q, d]] -> transpose
                base = 24 * s
                nc.vector.tensor_scalar(rhs24T[:, base:base + 8], expA[:, 8 * s:8 * s + 8],
                                        rd8[:, s:s + 1], None, OP.mult)
                nc.vector.tensor_scalar(rhs24T[:, base + 8:base + 16], eye[0:8, 0:8],
                                        rqd[:, s:s + 1], None, OP.mult)
                nc.vector.tensor_copy(rhs24T[:, base + 16:base + 24], eye[0:8, 0:8])
                pR = psM[0:24, 156 + 8 * s:156 + 8 * s + 8]
                nc.tensor.transpose(pR, rhs24T[:, base:base + 24], eye[0:8, 0:8])
                nc.vector.tensor_copy(rhs24[s][:], pR)

        def finals(s):
            for b in range(NB):
                fslot = psM[0:128, 172 + 8 * (b % 2):172 + 8 * (b % 2) + 8]
                nc.tensor.matmul(fslot, CM[s][:, 128 * b:128 * b + 128], rhs24[s][:],
                                 start=True, stop=True)
                o = OUTT[:, 16 * b + 8 * s:16 * b + 8 * s + 8]
                nc.vector.scalar_tensor_tensor(o, spatT[:, 16 * b + 8 * s:16 * b + 8 * s + 8],
                                               b_col(b, s),
                                               w1[:, 16 * b + 8 * s:16 * b + 8 * s + 8],
                                               OP.mult, OP.add)
                nc.vector.tensor_tensor(o, o, fslot, OP.add)
                tslot = psM[0:8, 192 + 128 * (b % 2):192 + 128 * (b % 2) + 128]
                nc.tensor.transpose(tslot, o, eye[:, :])
                nc.vector.tensor_copy(fin[s][:, 128 * b:128 * b + 128], tslot)
            nc.sync.dma_start(outs[s][:], fin[s][:])

        channel_path()
        phase_b(0)
        finals(0)
        phase_b(1)
        finals(1)


_PROGRAMS = {}


def _program(apply_wb=False):
    if apply_wb not in _PROGRAMS:
        _PROGRAMS[apply_wb] = build_program(apply_wb)
    return _PROGRAMS[apply_wb]


def kernel(F, P, norm_weight, norm_bias):
    from concourse.bass_utils import run_bass_kernel_spmd
    w = np.asarray(norm_weight, np.float32)
    b = np.asarray(norm_bias, np.float32)
    apply_wb = not (np.all(w == 1.0) and np.all(b == 0.0))
    nc = _program(apply_wb)
    maps = make_inmaps(F, P, norm_weight, norm_bias)
    res = run_bass_kernel_spmd(nc, maps, core_ids=list(range(8)), trace=False)
    return assemble(res.results)


# revision 5
# speedup vs baseline: 1.3011x; 1.0621x over previous
"""Trainium2 Bass kernel for nn_BRC_62715112457019 (sparse_attention), v2.

Head-parallel across 8 cores (core c = head c, both samples). Pixel-major
phase A (per-pixel stats/masks live on partitions -> no broadcast DMAs, tiny
128-wide DVE ops), fp8 DoubleRow phase B (QK^T and AV at 2 fp8 MACs/cycle),
per-qchunk transposed epilogue (no row-broadcasts), overlapped channel-attn
path and output writeback.

Pixel blocking: block b in [0,18) covers pixels [128b, 128b+128). Pixel-major
tiles are [128, 18*K] with column group b. Channel-major tensors ([8|16, HW])
are produced/consumed via PE transposes per block.
"""
import sys
for _p in ('/opt/trn_rl_repo', '/opt/pypackages'):
    if _p not in sys.path:
        sys.path.insert(0, _p)
import numpy as np
import ml_dtypes
from contextlib import ExitStack

import concourse.bass as bass
import concourse.bacc as bacc
import concourse.tile as tile
from concourse import mybir

dt = mybir.dt
F32 = dt.float32
BF16 = dt.bfloat16
FP8 = dt.float8e4
AF = mybir.ActivationFunctionType
OP = mybir.AluOpType
DR = mybir.MatmulPerfMode.DoubleRow

HW = 2304
NB = 18                       # 128-pixel blocks
CHUNKS = [(0, 512), (512, 512), (1024, 512), (1536, 512), (2048, 256)]
GRP = 3                       # logit blocks per exp group
BF = ml_dtypes.bfloat16
F8 = ml_dtypes.float8_e4m3fn


def host_constants(w8, b8):
    eye = np.eye(128, dtype=np.float32)
    eyeb = np.eye(128, dtype=BF)
    selsum = np.zeros((128, 5 * 37), BF)
    for c in range(5):
        selsum[0:64, 37 * c + c] = 1.0
        selsum[64:128, 37 * c + 32 + c] = 1.0
    wb16 = np.zeros((16, 2), np.float32)
    wb16[0:8, 0] = w8
    wb16[8:16, 0] = w8
    wb16[0:8, 1] = b8
    wb16[8:16, 1] = b8
    ones16 = np.ones((16, 1), np.float32)
    rep = np.zeros((6, 48), np.float32)
    for m in range(3):          # fg, bb, b
        for s in range(2):
            rep[2 * m + s, 16 * m + 8 * s:16 * m + 8 * s + 8] = 1.0
    return {"eye": eye, "eyeb": eyeb, "selsum": selsum, "wb16": wb16,
            "ones16": ones16, "rep": rep}


def make_inmaps(F, P, norm_weight, norm_bias):
    F = np.asarray(F, np.float32).reshape(2, 64, HW)
    P = np.asarray(P, np.float32).reshape(2, 48, 48)
    w = np.asarray(norm_weight, np.float32)
    b = np.asarray(norm_bias, np.float32)
    maps = []
    for c in range(8):
        m = host_constants(w[8 * c:8 * c + 8], b[8 * c:8 * c + 8])
        for n in range(2):
            m[f"Fb{n}"] = np.ascontiguousarray(F[n].astype(BF))
            m[f"F8_{n}"] = np.ascontiguousarray(F[n, 8 * c:8 * c + 8])
            m[f"P{n}"] = np.ascontiguousarray(P[n])
        maps.append(m)
    return maps


def assemble(results):
    out = np.empty((2, 64, 48, 48), np.float32)
    for c in range(8):
        for n in range(2):
            out[n, 8 * c:8 * c + 8] = results[c][f"out{n}"].reshape(8, 48, 48)
    return out


def build_program(apply_wb):
    nc = bacc.Bacc("TRN2", target_bir_lowering=False, debug=False)
    ins = {}
    for n in range(2):
        ins[f"Fb{n}"] = nc.dram_tensor(f"Fb{n}", [64, HW], BF16, kind="ExternalInput").ap()
        ins[f"F8_{n}"] = nc.dram_tensor(f"F8_{n}", [8, HW], F32, kind="ExternalInput").ap()
        ins[f"P{n}"] = nc.dram_tensor(f"P{n}", [48, 48], F32, kind="ExternalInput").ap()
    ins["eye"] = nc.dram_tensor("eye", [128, 128], F32, kind="ExternalInput").ap()
    ins["eyeb"] = nc.dram_tensor("eyeb", [128, 128], BF16, kind="ExternalInput").ap()
    ins["selsum"] = nc.dram_tensor("selsum", [128, 185], BF16, kind="ExternalInput").ap()
    ins["wb16"] = nc.dram_tensor("wb16", [16, 2], F32, kind="ExternalInput").ap()
    ins["ones16"] = nc.dram_tensor("ones16", [16, 1], F32, kind="ExternalInput").ap()
    ins["rep"] = nc.dram_tensor("rep", [6, 48], F32, kind="ExternalInput").ap()
    outs = [nc.dram_tensor(f"out{n}", [8, HW], F32, kind="ExternalOutput").ap() for n in range(2)]

    with tile.TileContext(nc) as tc:
        with ExitStack() as ctx:
            _body(ctx, tc, nc, ins, outs, apply_wb)
    nc.compile()
    return nc


# sobel slot indices (pairs of 50 cols: sample0|sample1, rows 0:48)
S_P50, S_PM, S_A1, S_TMP, S_B1, S_A1P, S_B1P, S_TCOL, S_GXT, S_GYT, S_M1, S_M2, \
    S_STT, S_BTM, S_BHW, S_FG, S_BG, S_BB = range(18)


def _body(ctx, tc, nc, ins, outs, apply_wb):
    pers = ctx.enter_context(tc.tile_pool(name="pers", bufs=1))
    sm = ctx.enter_context(tc.tile_pool(name="sm", bufs=1))

    # ---- persistent tiles ----
    eye = pers.tile([128, 128], F32, tag="eye")
    eyeb = pers.tile([128, 128], BF16, tag="eyeb")
    selsum = pers.tile([128, 185], BF16, tag="selsum")
    wb16 = pers.tile([16, 2], F32, tag="wb16")
    ones16 = pers.tile([16, 1], F32, tag="ones16")
    consts = pers.tile([128, 2], F32, tag="consts")     # col0 = eps
    F128 = pers.tile([128, HW], BF16, tag="F128")
    Fsq = pers.tile([128, HW], BF16, tag="Fsq")
    F16 = pers.tile([16, HW], F32, tag="F16")
    FnT = pers.tile([128, 288], F32, tag="FnT")         # 16b+8s+d
    qT = pers.tile([128, 288], F32, tag="qT")
    NRM = pers.tile([128, 36], F32, tag="NRM")          # 2b+s
    RQB = pers.tile([128, 72], F32, tag="RQB")          # [0:36] sqrt, [36:72] recip
    MT = pers.tile([128, 296], F32, tag="MT")           # 74j+37t+32s+c
    MK3 = pers.tile([128, 18 * 48], F32, tag="MK3")     # 48b+{fg16,bb16,b16}
    mrows = pers.tile([6, HW], F32, tag="mrows")
    rep = pers.tile([6, 48], F32, tag="rep")
    qcm16 = pers.tile([16, HW], BF16, tag="qcm16")
    qcm1 = pers.tile([8, HW], BF16, tag="qcm1")
    trTav = [pers.tile([128, 288], FP8, tag=f"trTav{s}", name=f"trTav{s}") for s in range(2)]
    bfg24 = pers.tile([128, 864], F32, tag="bfg24")    # [b][s][fg8|bb8|Fn8]
    CM = [pers.tile([24, HW], BF16, tag=f"CM{s}", name=f"CM{s}") for s in range(2)]
    Sall = pers.tile([128, 2 * NB * 512], FP8, tag="Sall")
    w1 = pers.tile([128, 288], F32, tag="w1")
    spatT = pers.tile([128, 288], F32, tag="spatT")
    OUTT = pers.tile([128, 288], F32, tag="OUTT")
    fin = [pers.tile([8, HW], F32, tag=f"fin{s}", name=f"fin{s}") for s in range(2)]
    rc = pers.tile([128, 8], F32, tag="rc")             # epilogue denominators
    sobm = pers.tile([48, 100 * 18], F32, tag="sobm")
    stm = pers.tile([37, 2560], F32, tag="stm")
    sq = pers.tile([128, 16], F32, tag="sq")
    # channel path smalls
    msk = pers.tile([16, 32], F32, tag="msk")
    r16f = pers.tile([16, 4], F32, tag="r16f")          # [0:2] sqrt, [2:4]=1/max(sqrt,..); col s
    rqd = pers.tile([8, 2], F32, tag="rqd")             # rq relocated to base 0
    A1 = pers.tile([8, 16], F32, tag="A1")
    A2 = pers.tile([8, 16], F32, tag="A2")
    expA = pers.tile([8, 16], F32, tag="expA")
    eden = pers.tile([8, 2], F32, tag="eden")
    rd8 = pers.tile([8, 2], F32, tag="rd8")
    rhs24T = pers.tile([8, 48], F32, tag="rhs24T")      # 24s col-block
    rhs24 = [pers.tile([24, 8], BF16, tag=f"rhs24_{s}", name=f"rhs24_{s}") for s in range(2)]
    WT = pers.tile([128, 16], F32, tag="WT") if apply_wb else None
    BT = pers.tile([128, 16], F32, tag="BT") if apply_wb else None

    def mu_col(b, s):
        return MT[:, 74 * (b % 4) + 32 * s + b // 4: 74 * (b % 4) + 32 * s + b // 4 + 1]

    def rs_col(b, s):
        o = 74 * (b % 4) + 37 + 32 * s + b // 4
        return MT[:, o:o + 1]

    mk3v = MK3[:].rearrange("p (b c) -> p b c", c=48)

    def mkv(m, s):
        return mk3v[:, :, 16 * m + 8 * s:16 * m + 8 * s + 8]

    fnv = FnT[:].rearrange("p (b c) -> p b c", c=16)
    w1v = w1[:].rearrange("p (b c) -> p b c", c=16)

    def fnsv(s):
        return fnv[:, :, 8 * s:8 * s + 8]

    def fnt(b, s):
        return FnT[:, 16 * b + 8 * s: 16 * b + 8 * s + 8]

    # =============== Phase A ===============
    with tc.tile_pool(name="psA", bufs=2, space="PSUM") as psA, \
         tc.tile_pool(name="psT", bufs=3, space="PSUM") as psT:

        nc.sync.dma_start(eye[:], ins["eye"])
        nc.sync.dma_start(eyeb[:], ins["eyeb"])
        nc.sync.dma_start(selsum[:], ins["selsum"])
        nc.sync.dma_start(wb16[:], ins["wb16"])
        nc.sync.dma_start(ones16[:], ins["ones16"])
        nc.sync.dma_start(rep[:], ins["rep"])
        nc.vector.memset(consts[:, 0:1], 1e-5)
        for s in range(2):
            nc.vector.memset(trTav[s][:], 0.0)
        for off, w in CHUNKS:
            nc.sync.dma_start(F128[0:64, off:off + w], ins["Fb0"][:, off:off + w])
            nc.sync.dma_start(F128[64:128, off:off + w], ins["Fb1"][:, off:off + w])
        nc.sync.dma_start(F16[0:8, :], ins["F8_0"])
        nc.sync.dma_start(F16[8:16, :], ins["F8_1"])

        # ---- masks (sobel in image space, batched samples in free dim) ----
        sv = sobm[:].rearrange("p (i s c) -> p i s c", s=2, c=50)

        def slot(i, r=(1, 49)):
            return sv[:, i, :, r[0]:r[1]]

        nc.gpsimd.memset(sobm[:, 0:200], 0.0)
        nc.sync.dma_start(slot(S_P50)[:, 0, :], ins["P0"])
        nc.sync.dma_start(slot(S_P50)[:, 1, :], ins["P1"])
        nc.scalar.activation(slot(S_PM), slot(S_P50), AF.Sigmoid)
        # exp table warmup (avoid mid-phase-B table load)
        nc.scalar.activation(sq[0:1, 0:1], consts[0:1, 0:1], AF.Exp)
        Pm0 = sv[:, S_PM]
        nc.vector.tensor_tensor(slot(S_A1), Pm0[:, :, 0:48], Pm0[:, :, 2:50], OP.subtract)
        nc.vector.tensor_tensor(slot(S_TMP), Pm0[:, :, 0:48], Pm0[:, :, 2:50], OP.add)
        nc.vector.scalar_tensor_tensor(slot(S_B1), Pm0[:, :, 1:49], 2.0, slot(S_TMP),
                                       OP.mult, OP.add)
        nc.gpsimd.memset(sobm[:, 100 * S_A1P:100 * S_A1P + 200], 0.0)
        for s in range(2):
            pt1 = psT.tile([128, 512], F32, tag="psT", name=f"pt1_{s}")
            nc.tensor.transpose(pt1[0:48, 0:48], slot(S_A1)[:, s, :], eye[0:48, 0:48])
            nc.vector.tensor_copy(slot(S_A1P)[:, s, :], pt1[0:48, 0:48])
            pt2 = psT.tile([128, 512], F32, tag="psT", name=f"pt2_{s}")
            nc.tensor.transpose(pt2[0:48, 0:48], slot(S_B1)[:, s, :], eye[0:48, 0:48])
            nc.vector.tensor_copy(slot(S_B1P)[:, s, :], pt2[0:48, 0:48])
        A1p = sv[:, S_A1P]
        B1p = sv[:, S_B1P]
        nc.vector.tensor_tensor(slot(S_TCOL), A1p[:, :, 0:48], A1p[:, :, 2:50], OP.add)
        nc.vector.scalar_tensor_tensor(slot(S_GXT), A1p[:, :, 1:49], 2.0, slot(S_TCOL),
                                       OP.mult, OP.add)
        nc.vector.tensor_tensor(slot(S_GYT), B1p[:, :, 0:48], B1p[:, :, 2:50], OP.subtract)
        nc.vector.tensor_tensor(slot(S_M1), slot(S_GXT), slot(S_GXT), OP.mult)
        nc.vector.tensor_tensor(slot(S_M2), slot(S_GYT), slot(S_GYT), OP.mult)
        nc.vector.tensor_tensor(slot(S_STT), slot(S_M1), slot(S_M2), OP.add)
        nc.vector.tensor_scalar(slot(S_BTM), slot(S_STT), 0.0, None, OP.is_gt)
        for s in range(2):
            pt3 = psT.tile([128, 512], F32, tag="psT", name=f"pt3_{s}")
            nc.tensor.transpose(pt3[0:48, 0:48], slot(S_BTM)[:, s, :], eye[0:48, 0:48])
            nc.vector.tensor_copy(slot(S_BHW)[:, s, :], pt3[0:48, 0:48])
        nc.vector.tensor_scalar(slot(S_FG), slot(S_P50), 0.0, None, OP.is_gt)
        nc.vector.tensor_scalar(slot(S_BG), slot(S_P50), 0.0, None, OP.is_lt)
        nc.vector.scalar_tensor_tensor(slot(S_BB), slot(S_BG), 1.0, slot(S_BHW),
                                       OP.mult, OP.max)
        # flatten masks to rows, then per-block transposes -> MKT
        for m, si in ((0, S_FG), (2, S_BB), (4, S_BHW)):
            for s in range(2):
                nc.sync.dma_start(mrows[m + s:m + s + 1, :], slot(si)[:, s, :])
        for b in range(NB):
            pm = psT.tile([128, 512], F32, tag="psT", name=f"pm{b}")
            nc.tensor.matmul(pm[0:128, 0:48], mrows[:, 128 * b:128 * b + 128],
                             rep[:], start=True, stop=True)
            nc.vector.tensor_copy(MK3[:, 48 * b:48 * b + 48], pm[0:128, 0:48])

        # ---- LayerNorm stats (channel-major PE reduction) ----
        for off, w in CHUNKS:
            nc.vector.tensor_tensor(Fsq[:, off:off + w], F128[:, off:off + w],
                                    F128[:, off:off + w], OP.mult)
        psumsA = psA.tile([128, 512], F32, tag="psA", name="psumsA")
        psumsB = psA.tile([128, 512], F32, tag="psA", name="psumsB")
        for c, (off, w) in enumerate(CHUNKS):
            nc.tensor.matmul(psumsA[0:37, 0:w], selsum[:, 37 * c:37 * c + 37],
                             F128[:, off:off + w], start=(c == 0), stop=(c == 4))
            nc.tensor.matmul(psumsB[0:37, 0:w], selsum[:, 37 * c:37 * c + 37],
                             Fsq[:, off:off + w], start=(c == 0), stop=(c == 4))
        s2 = stm[:, 0:512]
        varT = stm[:, 512:1024]
        sd = stm[:, 1024:1536]
        rstd = stm[:, 1536:2048]
        mu = stm[:, 2048:2560]
        nc.scalar.activation(s2, psumsA[0:37, :], AF.Square, scale=0.125)
        nc.vector.scalar_tensor_tensor(varT, psumsB[0:37, :], 1.0, s2, OP.mult, OP.subtract)
        nc.scalar.activation(sd, varT, AF.Sqrt, bias=consts[0:37, 0:1], scale=1.0 / 64.0)
        nc.vector.reciprocal(rstd, sd)
        nc.vector.tensor_scalar(mu, psumsA[0:37, :], 1.0 / 64.0, None, OP.mult)
        # stats -> pixel-major MT via transposes
        for j in range(4):
            for t, src in ((0, mu), (1, rstd)):
                pst = psT.tile([128, 512], F32, tag="psT", name=f"pst{j}_{t}")
                nc.tensor.transpose(pst[0:128, 0:37], src[:, 128 * j:128 * j + 128],
                                    eye[0:37, 0:37])
                nc.vector.tensor_copy(MT[:, 74 * j + 37 * t:74 * j + 37 * t + 37],
                                      pst[0:128, 0:37])

        if apply_wb:
            # broadcast per-channel w/b across partitions via ones-matmul
            pw = psT.tile([128, 512], F32, tag="psT", name="pw")
            nc.tensor.transpose(pw[0:2, 0:16], wb16[:], eye[0:16, 0:16])
            wbrow = sm.tile([2, 16], F32, tag="wbrow")
            nc.vector.tensor_copy(wbrow[:], pw[0:2, 0:16])
            onesr = sm.tile([1, 128], F32, tag="onesr")
            nc.vector.memset(onesr[:], 1.0)
            pw2 = psT.tile([128, 512], F32, tag="psT", name="pw2")
            nc.tensor.matmul(pw2[0:128, 0:16], onesr[:], wbrow[0:1, :], start=True, stop=True)
            nc.vector.tensor_copy(WT[:], pw2[0:128, 0:16])
            pw3 = psT.tile([128, 512], F32, tag="psT", name="pw3")
            nc.tensor.matmul(pw3[0:128, 0:16], onesr[:], wbrow[1:2, :], start=True, stop=True)
            nc.vector.tensor_copy(BT[:], pw3[0:128, 0:16])

        # ---- per-block: Fn_T, norms, q_T ----
        for b in range(NB):
            pF = psT.tile([128, 512], F32, tag="psT", name=f"pF{b}")
            nc.tensor.transpose(pF[0:128, 0:16], F16[:, 128 * b:128 * b + 128],
                                eye[0:16, 0:16])
            for s in range(2):
                nc.vector.tensor_scalar(fnt(b, s), pF[0:128, 8 * s:8 * s + 8],
                                        mu_col(b, s), rs_col(b, s), OP.subtract, OP.mult)
            if apply_wb:
                nc.vector.tensor_tensor(FnT[:, 16 * b:16 * b + 16],
                                        FnT[:, 16 * b:16 * b + 16], WT[:], OP.mult)
                nc.vector.tensor_tensor(FnT[:, 16 * b:16 * b + 16],
                                        FnT[:, 16 * b:16 * b + 16], BT[:], OP.add)
            for s in range(2):
                nc.vector.scalar_tensor_tensor(sq[:, 8 * s:8 * s + 8], fnt(b, s), 1.0,
                                               fnt(b, s), OP.mult, OP.mult,
                                               accum_out=NRM[:, 2 * b + s:2 * b + s + 1])
        nc.scalar.activation(RQB[:, 0:36], NRM[:], AF.Sqrt)
        nc.vector.tensor_scalar(RQB[:, 0:36], RQB[:, 0:36], 1e-12, None, OP.max)
        nc.vector.reciprocal(RQB[:, 36:72], RQB[:, 0:36])
        for b in range(NB):
            for s in range(2):
                nc.vector.tensor_scalar(qT[:, 16 * b + 8 * s:16 * b + 8 * s + 8],
                                        fnt(b, s), RQB[:, 36 + 2 * b + s:37 + 2 * b + s],
                                        None, OP.mult)
            pQ = psT.tile([128, 512], F32, tag="psT", name=f"pQ{b}")
            nc.tensor.transpose(pQ[0:16, 0:128], qT[:, 16 * b:16 * b + 16], eye[:, :])
            nc.scalar.activation(qcm16[:, 128 * b:128 * b + 128],
                                 pQ[0:16, 0:128], AF.Copy)
        nc.sync.dma_start(qcm1[:], qcm16[8:16, :])

        # ---- masked features: batched full-tile ops over 3D views ----
        b24 = bfg24[:].rearrange("p (b s c) -> p b s c", s=2, c=24)
        tvv = [trTav[s][:].rearrange("p (b c) -> p b c", c=16) for s in range(2)]
        for s in range(2):
            nc.vector.tensor_tensor(tvv[s][:, :, 0:8], fnsv(s), mkv(0, s), OP.mult)
            nc.vector.tensor_copy(tvv[s][:, :, 8:9], mk3v[:, :, 8 * s:8 * s + 1])
            nc.vector.tensor_tensor(b24[:, :, s, 0:8], fnsv(s), mkv(0, s), OP.mult)
            nc.vector.tensor_tensor(b24[:, :, s, 8:16], fnsv(s), mkv(1, s), OP.mult)
            nc.vector.tensor_copy(b24[:, :, s, 16:24], fnsv(s))
        for b in range(NB):
            for s in range(2):
                pC = psT.tile([128, 512], F32, tag="psT", name=f"pC{b}_{s}")
                nc.tensor.transpose(pC[0:24, 0:128],
                                    bfg24[:, 48 * b + 24 * s:48 * b + 24 * s + 24],
                                    eye[:, :])
                nc.vector.tensor_copy(CM[s][:, 128 * b:128 * b + 128],
                                      pC[0:24, 0:128])
        # w1 = Fn + b*(q - Fn)   (final out = w1 + Fn + b*spat + Fch + qc)
        nc.vector.tensor_tensor(w1[:], qT[:], FnT[:], OP.subtract)
        for s in range(2):
            nc.vector.tensor_tensor(w1v[:, :, 8 * s:8 * s + 8], w1v[:, :, 8 * s:8 * s + 8],
                                    mkv(2, s), OP.mult)
        nc.vector.tensor_tensor(w1[:], w1[:], FnT[:], OP.add)

    # =============== Phase B + channel path + finals ===============
    Sv = Sall[:].rearrange("p (t b x) -> p t b x", t=2, b=NB)
    tv = [trTav[s][:].rearrange("p (pb i c) -> p pb i c", i=2, c=16) for s in range(2)]

    with tc.tile_pool(name="psL", bufs=2, space="PSUM") as psL, \
         tc.tile_pool(name="psO", bufs=1, space="PSUM") as psO, \
         tc.tile_pool(name="psM", bufs=1, space="PSUM") as psMp, \
         tc.tile_pool(name="sS", bufs=2) as sS:
        psM = psMp.tile([128, 512], F32, tag="psM")
        kslot = [0]

        def phase_b(s):
            qsrc = qcm16[0:8, :] if s == 0 else qcm1[:]
            for jc, (joff, jw) in enumerate(CHUNKS):
                Sb = Sv[:, jc % 2]
                psOt = psO.tile([128, 512], F32, tag="psO", name=f"psO{s}_{jc}")

                def logits(g):
                    Lg = psL.tile([128, GRP * 512], F32, tag="L", name=f"L{s}_{jc}_{g}")
                    Lv = Lg[:].rearrange("p (i x) -> p i x", i=GRP)
                    for i in range(GRP):
                        b = GRP * g + i
                        nc.tensor.matmul(Lv[:, i, 0:jw], qsrc[:, 128 * b:128 * b + 128],
                                         qsrc[:, joff:joff + jw],
                                         start=True, stop=True)
                    nc.scalar.activation(Sb[:, GRP * g:GRP * g + GRP, 0:jw],
                                         Lv[:, :, 0:jw], AF.Exp)

                def av(pb):
                    nc.tensor.matmul(psOt[0:16, 0:jw], tv[s][:, pb], Sb[:, 2 * pb:2 * pb + 2, 0:jw],
                                     start=(pb == 0), stop=(pb == 8), perf_mode=DR)

                sched = {1: [0], 2: [1, 2], 3: [3], 4: [4, 5], 5: [6]}
                logits(0)
                for g in range(1, 6):
                    logits(g)
                    for pb in sched[g]:
                        av(pb)
                av(7)
                av(8)
                # transposed epilogue: spat = num/den, pixel-major
                avs = sS.tile([9, 512], F32, tag="avs", name=f"avs{s}_{jc}")
                nc.vector.tensor_copy(avs[:, 0:jw], psOt[0:9, 0:jw])
                for j in range(jw // 128):
                    b = 4 * jc + j
                    k = kslot[0] % 8
                    kslot[0] += 1
                    pslot = psM[0:128, 9 * k:9 * k + 9]
                    nc.tensor.transpose(pslot, avs[:, 128 * j:128 * j + 128], eye[0:9, 0:9])
                    nc.vector.reciprocal(rc[:, k:k + 1], pslot[:, 8:9])
                    nc.vector.tensor_scalar(spatT[:, 16 * b + 8 * s:16 * b + 8 * s + 8],
                                            pslot[:, 0:8], rc[:, k:k + 1], None, OP.mult)

        def channel_path():
            # per-sample Gram accumulation: psum16 = [fg|bb].T @ [fg|bb] over pixels
            ps16 = [psM[0:16, 72 + 16 * s:72 + 16 * s + 16] for s in range(2)]
            for s in range(2):
                for b in range(NB):
                    ap = bfg24[:, 48 * b + 24 * s:48 * b + 24 * s + 16]
                    nc.tensor.matmul(ps16[s], ap, ap,
                                     start=(b == 0), stop=(b == NB - 1))
            for s in range(2):
                nc.vector.tensor_tensor(msk[:, 16 * s:16 * s + 16], ps16[s],
                                        eye[0:16, 0:16], OP.mult)
                pd = psM[0:16, 104 + 2 * s:104 + 2 * s + 1]
                nc.tensor.matmul(pd, msk[:, 16 * s:16 * s + 16], ones16[:],
                                 start=True, stop=True)
                nc.scalar.activation(r16f[:, s:s + 1], pd, AF.Sqrt)
            nc.vector.tensor_scalar(r16f[:, 0:2], r16f[:, 0:2], 1e-12, None, OP.max)
            nc.vector.reciprocal(r16f[:, 2:4], r16f[:, 0:2])
            # rq (rows 8:16 of r16f) relocated to base 0
            nc.sync.dma_start(rqd[:], r16f[8:16, 2:4])
            for s in range(2):
                # Gram is symmetric: G^T[k, q] = Gram[0:8, 8:16] (fg rows, bb cols)
                nc.vector.tensor_scalar(A1[:, 8 * s:8 * s + 8],
                                        ps16[s][0:8, 8:16],
                                        r16f[0:8, 2 + s:3 + s], None, OP.mult)
                pA = psM[0:8, 140 + 8 * s:140 + 8 * s + 8]
                nc.tensor.transpose(pA, A1[:, 8 * s:8 * s + 8], eye[0:8, 0:8])
                nc.vector.tensor_scalar(A2[:, 8 * s:8 * s + 8], pA, rqd[:, s:s + 1],
                                        None, OP.mult)
                nc.scalar.activation(expA[:, 8 * s:8 * s + 8], A2[:, 8 * s:8 * s + 8],
                                     AF.Exp, accum_out=eden[:, s:s + 1])
                nc.vector.reciprocal(rd8[:, s:s + 1], eden[:, s:s + 1])
                # rhs24T[q, :] = [attn_c[q, k], diag(rq)[q, k], I8[q, d]] -> transpose
                base = 24 * s
                nc.vector.tensor_scalar(rhs24T[:, base:base + 8], expA[:, 8 * s:8 * s + 8],
                                        rd8[:, s:s + 1], None, OP.mult)
                nc.vector.tensor_scalar(rhs24T[:, base + 8:base + 16], eye[0:8, 0:8],
                                        rqd[:, s:s + 1], None, OP.mult)
                nc.vector.tensor_copy(rhs24T[:, base + 16:base + 24], eye[0:8, 0:8])
                pR = psM[0:24, 156 + 8 * s:156 + 8 * s + 8]
                nc.tensor.transpose(pR, rhs24T[:, base:base + 24], eye[0:8, 0:8])
                nc.vector.tensor_copy(rhs24[s][:], pR)

        spv = spatT[:].rearrange("p (b c) -> p b c", c=16)
        ov = OUTT[:].rearrange("p (b c) -> p b c", c=16)

        def finals(s):
            nc.vector.tensor_tensor(ov[:, :, 8 * s:8 * s + 8], spv[:, :, 8 * s:8 * s + 8],
                                    mkv(2, s), OP.mult)
            nc.vector.tensor_tensor(ov[:, :, 8 * s:8 * s + 8], ov[:, :, 8 * s:8 * s + 8],
                                    w1v[:, :, 8 * s:8 * s + 8], OP.add)
            for b in range(NB):
                fslot = psM[0:128, 172 + 8 * (b % 2):172 + 8 * (b % 2) + 8]
                nc.tensor.matmul(fslot, CM[s][:, 128 * b:128 * b + 128], rhs24[s][:],
                                 start=True, stop=True)
                o = OUTT[:, 16 * b + 8 * s:16 * b + 8 * s + 8]
                nc.vector.tensor_tensor(o, o, fslot, OP.add)
                tslot = psM[0:8, 192 + 128 * (b % 2):192 + 128 * (b % 2) + 128]
                nc.tensor.transpose(tslot, o, eye[:, :])
                nc.vector.tensor_copy(fin[s][:, 128 * b:128 * b + 128], tslot)
            nc.sync.dma_start(outs[s][:], fin[s][:])

        channel_path()
        phase_b(0)
        finals(0)
        phase_b(1)
        finals(1)


_PROGRAMS = {}


def _program(apply_wb=False):
    if apply_wb not in _PROGRAMS:
        _PROGRAMS[apply_wb] = build_program(apply_wb)
    return _PROGRAMS[apply_wb]


def kernel(F, P, norm_weight, norm_bias):
    from concourse.bass_utils import run_bass_kernel_spmd
    w = np.asarray(norm_weight, np.float32)
    b = np.asarray(norm_bias, np.float32)
    apply_wb = not (np.all(w == 1.0) and np.all(b == 0.0))
    nc = _program(apply_wb)
    maps = make_inmaps(F, P, norm_weight, norm_bias)
    res = run_bass_kernel_spmd(nc, maps, core_ids=list(range(8)), trace=False)
    return assemble(res.results)


# revision 6
# speedup vs baseline: 1.3801x; 1.0608x over previous
"""Trainium2 Bass kernel for nn_BRC_62715112457019 (sparse_attention), v2.

Head-parallel across 8 cores (core c = head c, both samples). Pixel-major
phase A (per-pixel stats/masks live on partitions -> no broadcast DMAs, tiny
128-wide DVE ops), fp8 DoubleRow phase B (QK^T and AV at 2 fp8 MACs/cycle),
per-qchunk transposed epilogue (no row-broadcasts), overlapped channel-attn
path and output writeback.

Pixel blocking: block b in [0,18) covers pixels [128b, 128b+128). Pixel-major
tiles are [128, 18*K] with column group b. Channel-major tensors ([8|16, HW])
are produced/consumed via PE transposes per block.
"""
import sys
for _p in ('/opt/trn_rl_repo', '/opt/pypackages'):
    if _p not in sys.path:
        sys.path.insert(0, _p)
import numpy as np
import ml_dtypes
from contextlib import ExitStack

import concourse.bass as bass
import concourse.bacc as bacc
import concourse.tile as tile
from concourse import mybir

dt = mybir.dt
F32 = dt.float32
BF16 = dt.bfloat16
FP8 = dt.float8e4
AF = mybir.ActivationFunctionType
OP = mybir.AluOpType
DR = mybir.MatmulPerfMode.DoubleRow

HW = 2304
NB = 18                       # 128-pixel blocks
CHUNKS = [(0, 512), (512, 512), (1024, 512), (1536, 512), (2048, 256)]
GRP = 3                       # logit blocks per exp group
BF = ml_dtypes.bfloat16
F8 = ml_dtypes.float8_e4m3fn


def host_constants(w8, b8):
    eye = np.eye(128, dtype=np.float32)
    eyeb = np.eye(128, dtype=BF)
    selsum = np.zeros((128, 5 * 37), BF)
    for c in range(5):
        selsum[0:64, 37 * c + c] = 1.0
        selsum[64:128, 37 * c + 32 + c] = 1.0
    wb16 = np.zeros((16, 2), np.float32)
    wb16[0:8, 0] = w8
    wb16[8:16, 0] = w8
    wb16[0:8, 1] = b8
    wb16[8:16, 1] = b8
    ones16 = np.ones((16, 1), np.float32)
    rep = np.zeros((6, 48), np.float32)
    for m in range(3):          # fg, bb, b
        for s in range(2):
            rep[2 * m + s, 16 * m + 8 * s:16 * m + 8 * s + 8] = 1.0
    return {"eye": eye, "eyeb": eyeb, "selsum": selsum, "wb16": wb16,
            "ones16": ones16, "rep": rep}


def make_inmaps(F, P, norm_weight, norm_bias):
    F = np.asarray(F, np.float32).reshape(2, 64, HW)
    P = np.asarray(P, np.float32).reshape(2, 48, 48)
    w = np.asarray(norm_weight, np.float32)
    b = np.asarray(norm_bias, np.float32)
    maps = []
    for c in range(8):
        m = host_constants(w[8 * c:8 * c + 8], b[8 * c:8 * c + 8])
        for n in range(2):
            m[f"Fb{n}"] = np.ascontiguousarray(F[n].astype(BF))
            m[f"F8_{n}"] = np.ascontiguousarray(F[n, 8 * c:8 * c + 8])
            m[f"P{n}"] = np.ascontiguousarray(P[n])
        maps.append(m)
    return maps


def assemble(results):
    out = np.empty((2, 64, 48, 48), np.float32)
    for c in range(8):
        for n in range(2):
            out[n, 8 * c:8 * c + 8] = results[c][f"out{n}"].reshape(8, 48, 48)
    return out


def build_program(apply_wb):
    nc = bacc.Bacc("TRN2", target_bir_lowering=False, debug=False)
    ins = {}
    for n in range(2):
        ins[f"Fb{n}"] = nc.dram_tensor(f"Fb{n}", [64, HW], BF16, kind="ExternalInput").ap()
        ins[f"F8_{n}"] = nc.dram_tensor(f"F8_{n}", [8, HW], F32, kind="ExternalInput").ap()
        ins[f"P{n}"] = nc.dram_tensor(f"P{n}", [48, 48], F32, kind="ExternalInput").ap()
    ins["eye"] = nc.dram_tensor("eye", [128, 128], F32, kind="ExternalInput").ap()
    ins["eyeb"] = nc.dram_tensor("eyeb", [128, 128], BF16, kind="ExternalInput").ap()
    ins["selsum"] = nc.dram_tensor("selsum", [128, 185], BF16, kind="ExternalInput").ap()
    ins["wb16"] = nc.dram_tensor("wb16", [16, 2], F32, kind="ExternalInput").ap()
    ins["ones16"] = nc.dram_tensor("ones16", [16, 1], F32, kind="ExternalInput").ap()
    ins["rep"] = nc.dram_tensor("rep", [6, 48], F32, kind="ExternalInput").ap()
    outs = [nc.dram_tensor(f"out{n}", [8, HW], F32, kind="ExternalOutput").ap() for n in range(2)]

    with tile.TileContext(nc) as tc:
        with ExitStack() as ctx:
            _body(ctx, tc, nc, ins, outs, apply_wb)
    nc.compile()
    return nc


# sobel slot indices (pairs of 50 cols: sample0|sample1, rows 0:48)
S_P50, S_PM, S_A1, S_TMP, S_B1, S_A1P, S_B1P, S_TCOL, S_GXT, S_GYT, S_M1, S_M2, \
    S_STT, S_BTM, S_BHW, S_FG, S_BG, S_BB = range(18)


def _body(ctx, tc, nc, ins, outs, apply_wb):
    pers = ctx.enter_context(tc.tile_pool(name="pers", bufs=1))
    sm = ctx.enter_context(tc.tile_pool(name="sm", bufs=1))

    # ---- persistent tiles ----
    eye = pers.tile([128, 128], F32, tag="eye")
    eyeb = pers.tile([128, 128], BF16, tag="eyeb")
    selsum = pers.tile([128, 185], BF16, tag="selsum")
    wb16 = pers.tile([16, 2], F32, tag="wb16")
    ones16 = pers.tile([16, 1], F32, tag="ones16")
    consts = pers.tile([128, 2], F32, tag="consts")     # col0 = eps
    F128 = pers.tile([128, HW], BF16, tag="F128")
    Fsq = pers.tile([128, HW], BF16, tag="Fsq")
    F16 = pers.tile([16, HW], F32, tag="F16")
    FnT = pers.tile([128, 288], F32, tag="FnT")         # 16b+8s+d
    qT = pers.tile([128, 288], F32, tag="qT")
    NRM = pers.tile([128, 36], F32, tag="NRM")          # 2b+s
    RQB = pers.tile([128, 72], F32, tag="RQB")          # [0:36] sqrt, [36:72] recip
    MT = pers.tile([128, 296], F32, tag="MT")           # 74j+37t+32s+c
    MK3 = pers.tile([128, 18 * 48], F32, tag="MK3")     # 48b+{fg16,bb16,b16}
    mrows = pers.tile([6, HW], F32, tag="mrows")
    rep = pers.tile([6, 48], F32, tag="rep")
    qcm16 = pers.tile([16, HW], BF16, tag="qcm16")
    qcm1 = pers.tile([8, HW], BF16, tag="qcm1")
    trTav = [pers.tile([128, 288], FP8, tag=f"trTav{s}", name=f"trTav{s}") for s in range(2)]
    bfg24 = pers.tile([128, 864], F32, tag="bfg24")    # [b][s][fg8|bb8|Fn8]
    CM = [pers.tile([24, HW], BF16, tag=f"CM{s}", name=f"CM{s}") for s in range(2)]
    Sall = pers.tile([128, 2 * NB * 512], FP8, tag="Sall")
    w1 = pers.tile([128, 288], F32, tag="w1")
    spatT = pers.tile([128, 288], F32, tag="spatT")
    OUTT = pers.tile([128, 288], F32, tag="OUTT")
    fin = [pers.tile([8, HW], F32, tag=f"fin{s}", name=f"fin{s}") for s in range(2)]
    rc = pers.tile([128, 8], F32, tag="rc")             # epilogue denominators
    sobm = pers.tile([48, 100 * 18], F32, tag="sobm")
    stm = pers.tile([37, 2560], F32, tag="stm")
    sq = pers.tile([128, 16], F32, tag="sq")
    # channel path smalls
    msk = pers.tile([16, 32], F32, tag="msk")
    r16f = pers.tile([16, 4], F32, tag="r16f")          # [0:2] sqrt, [2:4]=1/max(sqrt,..); col s
    rqd = pers.tile([8, 2], F32, tag="rqd")             # rq relocated to base 0
    A1 = pers.tile([8, 16], F32, tag="A1")
    A2 = pers.tile([8, 16], F32, tag="A2")
    expA = pers.tile([8, 16], F32, tag="expA")
    eden = pers.tile([8, 2], F32, tag="eden")
    rd8 = pers.tile([8, 2], F32, tag="rd8")
    rhs24T = pers.tile([8, 48], F32, tag="rhs24T")      # 24s col-block
    rhs24 = [pers.tile([24, 8], BF16, tag=f"rhs24_{s}", name=f"rhs24_{s}") for s in range(2)]
    WT = pers.tile([128, 16], F32, tag="WT") if apply_wb else None
    BT = pers.tile([128, 16], F32, tag="BT") if apply_wb else None

    def mu_col(b, s):
        return MT[:, 74 * (b % 4) + 32 * s + b // 4: 74 * (b % 4) + 32 * s + b // 4 + 1]

    def rs_col(b, s):
        o = 74 * (b % 4) + 37 + 32 * s + b // 4
        return MT[:, o:o + 1]

    mk3v = MK3[:].rearrange("p (b c) -> p b c", c=48)

    def mkv(m, s):
        return mk3v[:, :, 16 * m + 8 * s:16 * m + 8 * s + 8]

    fnv = FnT[:].rearrange("p (b c) -> p b c", c=16)
    w1v = w1[:].rearrange("p (b c) -> p b c", c=16)

    def fnsv(s):
        return fnv[:, :, 8 * s:8 * s + 8]

    def fnt(b, s):
        return FnT[:, 16 * b + 8 * s: 16 * b + 8 * s + 8]

    # =============== Phase A ===============
    with tc.tile_pool(name="psA", bufs=2, space="PSUM") as psA, \
         tc.tile_pool(name="psT", bufs=3, space="PSUM") as psT:

        nc.sync.dma_start(eye[:], ins["eye"])
        nc.sync.dma_start(eyeb[:], ins["eyeb"])
        nc.sync.dma_start(selsum[:], ins["selsum"])
        nc.sync.dma_start(wb16[:], ins["wb16"])
        nc.sync.dma_start(ones16[:], ins["ones16"])
        nc.sync.dma_start(rep[:], ins["rep"])
        nc.vector.memset(consts[:, 0:1], 1e-5)
        for s in range(2):
            nc.vector.memset(trTav[s][:], 0.0)
        for off, w in CHUNKS:
            nc.sync.dma_start(F128[0:64, off:off + w], ins["Fb0"][:, off:off + w])
            nc.sync.dma_start(F128[64:128, off:off + w], ins["Fb1"][:, off:off + w])
        nc.sync.dma_start(F16[0:8, :], ins["F8_0"])
        nc.sync.dma_start(F16[8:16, :], ins["F8_1"])

        # ---- masks (sobel in image space, batched samples in free dim) ----
        sv = sobm[:].rearrange("p (i s c) -> p i s c", s=2, c=50)

        def slot(i, r=(1, 49)):
            return sv[:, i, :, r[0]:r[1]]

        nc.gpsimd.memset(sobm[:, 0:200], 0.0)
        nc.sync.dma_start(slot(S_P50)[:, 0, :], ins["P0"])
        nc.sync.dma_start(slot(S_P50)[:, 1, :], ins["P1"])
        nc.scalar.activation(slot(S_PM), slot(S_P50), AF.Sigmoid)
        # exp table warmup (avoid mid-phase-B table load)
        nc.scalar.activation(sq[0:1, 0:1], consts[0:1, 0:1], AF.Exp)
        Pm0 = sv[:, S_PM]
        nc.vector.tensor_tensor(slot(S_A1), Pm0[:, :, 0:48], Pm0[:, :, 2:50], OP.subtract)
        nc.vector.tensor_tensor(slot(S_TMP), Pm0[:, :, 0:48], Pm0[:, :, 2:50], OP.add)
        nc.vector.scalar_tensor_tensor(slot(S_B1), Pm0[:, :, 1:49], 2.0, slot(S_TMP),
                                       OP.mult, OP.add)
        nc.gpsimd.memset(sobm[:, 100 * S_A1P:100 * S_A1P + 200], 0.0)
        for s in range(2):
            pt1 = psT.tile([128, 512], F32, tag="psT", name=f"pt1_{s}")
            nc.tensor.transpose(pt1[0:48, 0:48], slot(S_A1)[:, s, :], eye[0:48, 0:48])
            nc.vector.tensor_copy(slot(S_A1P)[:, s, :], pt1[0:48, 0:48])
            pt2 = psT.tile([128, 512], F32, tag="psT", name=f"pt2_{s}")
            nc.tensor.transpose(pt2[0:48, 0:48], slot(S_B1)[:, s, :], eye[0:48, 0:48])
            nc.vector.tensor_copy(slot(S_B1P)[:, s, :], pt2[0:48, 0:48])
        A1p = sv[:, S_A1P]
        B1p = sv[:, S_B1P]
        nc.vector.tensor_tensor(slot(S_TCOL), A1p[:, :, 0:48], A1p[:, :, 2:50], OP.add)
        nc.vector.scalar_tensor_tensor(slot(S_GXT), A1p[:, :, 1:49], 2.0, slot(S_TCOL),
                                       OP.mult, OP.add)
        nc.vector.tensor_tensor(slot(S_GYT), B1p[:, :, 0:48], B1p[:, :, 2:50], OP.subtract)
        nc.vector.tensor_tensor(slot(S_M1), slot(S_GXT), slot(S_GXT), OP.mult)
        nc.vector.tensor_tensor(slot(S_M2), slot(S_GYT), slot(S_GYT), OP.mult)
        nc.vector.tensor_tensor(slot(S_STT), slot(S_M1), slot(S_M2), OP.add)
        nc.vector.tensor_scalar(slot(S_BTM), slot(S_STT), 0.0, None, OP.is_gt)
        for s in range(2):
            pt3 = psT.tile([128, 512], F32, tag="psT", name=f"pt3_{s}")
            nc.tensor.transpose(pt3[0:48, 0:48], slot(S_BTM)[:, s, :], eye[0:48, 0:48])
            nc.vector.tensor_copy(slot(S_BHW)[:, s, :], pt3[0:48, 0:48])
        nc.vector.tensor_scalar(slot(S_FG), slot(S_P50), 0.0, None, OP.is_gt)
        nc.vector.tensor_scalar(slot(S_BG), slot(S_P50), 0.0, None, OP.is_lt)
        nc.vector.scalar_tensor_tensor(slot(S_BB), slot(S_BG), 1.0, slot(S_BHW),
                                       OP.mult, OP.max)
        # flatten masks to rows, then per-block transposes -> MKT
        for m, si in ((0, S_FG), (2, S_BB), (4, S_BHW)):
            for s in range(2):
                nc.sync.dma_start(mrows[m + s:m + s + 1, :], slot(si)[:, s, :])
        for b in range(NB):
            pm = psT.tile([128, 512], F32, tag="psT", name=f"pm{b}")
            nc.tensor.matmul(pm[0:128, 0:48], mrows[:, 128 * b:128 * b + 128],
                             rep[:], start=True, stop=True)
            nc.vector.tensor_copy(MK3[:, 48 * b:48 * b + 48], pm[0:128, 0:48])

        # ---- LayerNorm stats (channel-major PE reduction) ----
        for off, w in CHUNKS:
            nc.vector.tensor_tensor(Fsq[:, off:off + w], F128[:, off:off + w],
                                    F128[:, off:off + w], OP.mult)
        psumsA = psA.tile([128, 512], F32, tag="psA", name="psumsA")
        psumsB = psA.tile([128, 512], F32, tag="psA", name="psumsB")
        for c, (off, w) in enumerate(CHUNKS):
            nc.tensor.matmul(psumsA[0:37, 0:w], selsum[:, 37 * c:37 * c + 37],
                             F128[:, off:off + w], start=(c == 0), stop=(c == 4))
            nc.tensor.matmul(psumsB[0:37, 0:w], selsum[:, 37 * c:37 * c + 37],
                             Fsq[:, off:off + w], start=(c == 0), stop=(c == 4))
        s2 = stm[:, 0:512]
        varT = stm[:, 512:1024]
        sd = stm[:, 1024:1536]
        rstd = stm[:, 1536:2048]
        mu = stm[:, 2048:2560]
        nc.scalar.activation(s2, psumsA[0:37, :], AF.Square, scale=0.125)
        nc.vector.scalar_tensor_tensor(varT, psumsB[0:37, :], 1.0, s2, OP.mult, OP.subtract)
        nc.scalar.activation(sd, varT, AF.Sqrt, bias=consts[0:37, 0:1], scale=1.0 / 64.0)
        nc.vector.reciprocal(rstd, sd)
        nc.vector.tensor_scalar(mu, psumsA[0:37, :], 1.0 / 64.0, None, OP.mult)
        # stats -> pixel-major MT via transposes
        for j in range(4):
            for t, src in ((0, mu), (1, rstd)):
                pst = psT.tile([128, 512], F32, tag="psT", name=f"pst{j}_{t}")
                nc.tensor.transpose(pst[0:128, 0:37], src[:, 128 * j:128 * j + 128],
                                    eye[0:37, 0:37])
                nc.vector.tensor_copy(MT[:, 74 * j + 37 * t:74 * j + 37 * t + 37],
                                      pst[0:128, 0:37])

        if apply_wb:
            # broadcast per-channel w/b across partitions via ones-matmul
            pw = psT.tile([128, 512], F32, tag="psT", name="pw")
            nc.tensor.transpose(pw[0:2, 0:16], wb16[:], eye[0:16, 0:16])
            wbrow = sm.tile([2, 16], F32, tag="wbrow")
            nc.vector.tensor_copy(wbrow[:], pw[0:2, 0:16])
            onesr = sm.tile([1, 128], F32, tag="onesr")
            nc.vector.memset(onesr[:], 1.0)
            pw2 = psT.tile([128, 512], F32, tag="psT", name="pw2")
            nc.tensor.matmul(pw2[0:128, 0:16], onesr[:], wbrow[0:1, :], start=True, stop=True)
            nc.vector.tensor_copy(WT[:], pw2[0:128, 0:16])
            pw3 = psT.tile([128, 512], F32, tag="psT", name="pw3")
            nc.tensor.matmul(pw3[0:128, 0:16], onesr[:], wbrow[1:2, :], start=True, stop=True)
            nc.vector.tensor_copy(BT[:], pw3[0:128, 0:16])

        # ---- per-block: Fn_T, norms, q_T ----
        for b in range(NB):
            pF = psT.tile([128, 512], F32, tag="psT", name=f"pF{b}")
            nc.tensor.transpose(pF[0:128, 0:16], F16[:, 128 * b:128 * b + 128],
                                eye[0:16, 0:16])
            for s in range(2):
                nc.vector.tensor_scalar(fnt(b, s), pF[0:128, 8 * s:8 * s + 8],
                                        mu_col(b, s), rs_col(b, s), OP.subtract, OP.mult)
            if apply_wb:
                nc.vector.tensor_tensor(FnT[:, 16 * b:16 * b + 16],
                                        FnT[:, 16 * b:16 * b + 16], WT[:], OP.mult)
                nc.vector.tensor_tensor(FnT[:, 16 * b:16 * b + 16],
                                        FnT[:, 16 * b:16 * b + 16], BT[:], OP.add)
            for s in range(2):
                nc.vector.scalar_tensor_tensor(sq[:, 8 * s:8 * s + 8], fnt(b, s), 1.0,
                                               fnt(b, s), OP.mult, OP.mult,
                                               accum_out=NRM[:, 2 * b + s:2 * b + s + 1])
        nc.scalar.activation(RQB[:, 0:36], NRM[:], AF.Sqrt)
        nc.vector.tensor_scalar(RQB[:, 0:36], RQB[:, 0:36], 1e-12, None, OP.max)
        nc.vector.reciprocal(RQB[:, 36:72], RQB[:, 0:36])
        for b in range(NB):
            for s in range(2):
                nc.vector.tensor_scalar(qT[:, 16 * b + 8 * s:16 * b + 8 * s + 8],
                                        fnt(b, s), RQB[:, 36 + 2 * b + s:37 + 2 * b + s],
                                        None, OP.mult)
            pQ = psT.tile([128, 512], F32, tag="psT", name=f"pQ{b}")
            nc.tensor.transpose(pQ[0:16, 0:128], qT[:, 16 * b:16 * b + 16], eye[:, :])
            nc.scalar.activation(qcm16[:, 128 * b:128 * b + 128],
                                 pQ[0:16, 0:128], AF.Copy)
        nc.sync.dma_start(qcm1[:], qcm16[8:16, :])

        # ---- masked features: batched full-tile ops over 3D views ----
        b24 = bfg24[:].rearrange("p (b s c) -> p b s c", s=2, c=24)
        tvv = [trTav[s][:].rearrange("p (b c) -> p b c", c=16) for s in range(2)]
        for s in range(2):
            nc.vector.tensor_tensor(tvv[s][:, :, 0:8], fnsv(s), mkv(0, s), OP.mult)
            nc.vector.tensor_copy(tvv[s][:, :, 8:9], mk3v[:, :, 8 * s:8 * s + 1])
            nc.vector.tensor_tensor(b24[:, :, s, 0:8], fnsv(s), mkv(0, s), OP.mult)
            nc.vector.tensor_tensor(b24[:, :, s, 8:16], fnsv(s), mkv(1, s), OP.mult)
            nc.vector.tensor_copy(b24[:, :, s, 16:24], fnsv(s))
        for b in range(NB):
            for s in range(2):
                pC = psT.tile([128, 512], F32, tag="psT", name=f"pC{b}_{s}")
                nc.tensor.transpose(pC[0:24, 0:128],
                                    bfg24[:, 48 * b + 24 * s:48 * b + 24 * s + 24],
                                    eye[:, :])
                nc.vector.tensor_copy(CM[s][:, 128 * b:128 * b + 128],
                                      pC[0:24, 0:128])
        # w1 = Fn + b*(q - Fn)   (final out = w1 + Fn + b*spat + Fch + qc)
        nc.vector.tensor_tensor(w1[:], qT[:], FnT[:], OP.subtract)
        for s in range(2):
            nc.vector.tensor_tensor(w1v[:, :, 8 * s:8 * s + 8], w1v[:, :, 8 * s:8 * s + 8],
                                    mkv(2, s), OP.mult)
        nc.vector.tensor_tensor(w1[:], w1[:], FnT[:], OP.add)

    # =============== Phase B + channel path + finals ===============
    Sv = Sall[:].rearrange("p (t b x) -> p t b x", t=2, b=NB)
    tv = [trTav[s][:].rearrange("p (pb i c) -> p pb i c", i=2, c=16) for s in range(2)]

    with tc.tile_pool(name="psL", bufs=2, space="PSUM") as psL, \
         tc.tile_pool(name="psO", bufs=1, space="PSUM") as psO, \
         tc.tile_pool(name="psM", bufs=1, space="PSUM") as psMp, \
         tc.tile_pool(name="sS", bufs=2) as sS:
        psM = psMp.tile([128, 512], F32, tag="psM")
        kslot = [0]

        def phase_b(s):
            qsrc = qcm16[0:8, :] if s == 0 else qcm1[:]
            for jc, (joff, jw) in enumerate(CHUNKS):
                Sb = Sv[:, jc % 2]
                psOt = psO.tile([128, 512], F32, tag="psO", name=f"psO{s}_{jc}")

                def logits(g):
                    Lg = psL.tile([128, GRP * 512], F32, tag="L", name=f"L{s}_{jc}_{g}")
                    Lv = Lg[:].rearrange("p (i x) -> p i x", i=GRP)
                    for i in range(GRP):
                        b = GRP * g + i
                        nc.tensor.matmul(Lv[:, i, 0:jw], qsrc[:, 128 * b:128 * b + 128],
                                         qsrc[:, joff:joff + jw],
                                         start=True, stop=True)
                    nc.scalar.activation(Sb[:, GRP * g:GRP * g + GRP, 0:jw],
                                         Lv[:, :, 0:jw], AF.Exp)

                def av(pb):
                    nc.tensor.matmul(psOt[0:16, 0:jw], tv[s][:, pb], Sb[:, 2 * pb:2 * pb + 2, 0:jw],
                                     start=(pb == 0), stop=(pb == 8), perf_mode=DR)

                sched = {1: [0], 2: [1, 2], 3: [3], 4: [4, 5], 5: [6]}
                logits(0)
                for g in range(1, 6):
                    logits(g)
                    for pb in sched[g]:
                        av(pb)
                av(7)
                av(8)
                # transposed epilogue: spat = num/den, pixel-major
                avs = sS.tile([9, 512], F32, tag="avs", name=f"avs{s}_{jc}")
                nc.vector.tensor_copy(avs[:, 0:jw], psOt[0:9, 0:jw])
                for j in range(jw // 128):
                    b = 4 * jc + j
                    k = kslot[0] % 8
                    kslot[0] += 1
                    pslot = psM[0:128, 9 * k:9 * k + 9]
                    nc.tensor.transpose(pslot, avs[:, 128 * j:128 * j + 128], eye[0:9, 0:9])
                    nc.vector.reciprocal(rc[:, k:k + 1], pslot[:, 8:9])
                    nc.vector.tensor_scalar(spatT[:, 16 * b + 8 * s:16 * b + 8 * s + 8],
                                            pslot[:, 0:8], rc[:, k:k + 1], None, OP.mult)
                finals_chunk(s, jc, joff, jw)

        def channel_path():
            # per-sample Gram accumulation: psum16 = [fg|bb].T @ [fg|bb] over pixels
            ps16 = [psM[0:16, 72 + 16 * s:72 + 16 * s + 16] for s in range(2)]
            for s in range(2):
                for b in range(NB):
                    ap = bfg24[:, 48 * b + 24 * s:48 * b + 24 * s + 16]
                    nc.tensor.matmul(ps16[s], ap, ap,
                                     start=(b == 0), stop=(b == NB - 1))
            for s in range(2):
                nc.vector.tensor_tensor(msk[:, 16 * s:16 * s + 16], ps16[s],
                                        eye[0:16, 0:16], OP.mult)
                pd = psM[0:16, 104 + 2 * s:104 + 2 * s + 1]
                nc.tensor.matmul(pd, msk[:, 16 * s:16 * s + 16], ones16[:],
                                 start=True, stop=True)
                nc.scalar.activation(r16f[:, s:s + 1], pd, AF.Sqrt)
            nc.vector.tensor_scalar(r16f[:, 0:2], r16f[:, 0:2], 1e-12, None, OP.max)
            nc.vector.reciprocal(r16f[:, 2:4], r16f[:, 0:2])
            # rq (rows 8:16 of r16f) relocated to base 0
            nc.sync.dma_start(rqd[:], r16f[8:16, 2:4])
            for s in range(2):
                # Gram is symmetric: G^T[k, q] = Gram[0:8, 8:16] (fg rows, bb cols)
                nc.vector.tensor_scalar(A1[:, 8 * s:8 * s + 8],
                                        ps16[s][0:8, 8:16],
                                        r16f[0:8, 2 + s:3 + s], None, OP.mult)
                pA = psM[0:8, 140 + 8 * s:140 + 8 * s + 8]
                nc.tensor.transpose(pA, A1[:, 8 * s:8 * s + 8], eye[0:8, 0:8])
                nc.vector.tensor_scalar(A2[:, 8 * s:8 * s + 8], pA, rqd[:, s:s + 1],
                                        None, OP.mult)
                nc.scalar.activation(expA[:, 8 * s:8 * s + 8], A2[:, 8 * s:8 * s + 8],
                                     AF.Exp, accum_out=eden[:, s:s + 1])
                nc.vector.reciprocal(rd8[:, s:s + 1], eden[:, s:s + 1])
                # rhs24T[q, :] = [attn_c[q, k], diag(rq)[q, k], I8[q, d]] -> transpose
                base = 24 * s
                nc.vector.tensor_scalar(rhs24T[:, base:base + 8], expA[:, 8 * s:8 * s + 8],
                                        rd8[:, s:s + 1], None, OP.mult)
                nc.vector.tensor_scalar(rhs24T[:, base + 8:base + 16], eye[0:8, 0:8],
                                        rqd[:, s:s + 1], None, OP.mult)
                nc.vector.tensor_copy(rhs24T[:, base + 16:base + 24], eye[0:8, 0:8])
                pR = psM[0:24, 156 + 8 * s:156 + 8 * s + 8]
                nc.tensor.transpose(pR, rhs24T[:, base:base + 24], eye[0:8, 0:8])
                nc.vector.tensor_copy(rhs24[s][:], pR)

        spv = spatT[:].rearrange("p (b c) -> p b c", c=16)
        ov = OUTT[:].rearrange("p (b c) -> p b c", c=16)

        def finals_chunk(s, jc, joff, jw):
            nb = jw // 128
            bs = slice(4 * jc, 4 * jc + nb)
            nc.vector.tensor_tensor(ov[:, bs, 8 * s:8 * s + 8], spv[:, bs, 8 * s:8 * s + 8],
                                    mkv(2, s)[:, bs, :], OP.mult)
            nc.vector.tensor_tensor(ov[:, bs, 8 * s:8 * s + 8], ov[:, bs, 8 * s:8 * s + 8],
                                    w1v[:, bs, 8 * s:8 * s + 8], OP.add)
            for b in range(4 * jc, 4 * jc + nb):
                fslot = psM[0:128, 172 + 8 * (b % 2):172 + 8 * (b % 2) + 8]
                nc.tensor.matmul(fslot, CM[s][:, 128 * b:128 * b + 128], rhs24[s][:],
                                 start=True, stop=True)
                o = OUTT[:, 16 * b + 8 * s:16 * b + 8 * s + 8]
                nc.vector.tensor_tensor(o, o, fslot, OP.add)
                tslot = psM[0:8, 192 + 128 * (b % 2):192 + 128 * (b % 2) + 128]
                nc.tensor.transpose(tslot, o, eye[:, :])
                nc.vector.tensor_copy(fin[s][:, 128 * b:128 * b + 128], tslot)
            nc.sync.dma_start(outs[s][:, joff:joff + jw], fin[s][:, joff:joff + jw])

        channel_path()
        phase_b(0)
        phase_b(1)


_PROGRAMS = {}


def _program(apply_wb=False):
    if apply_wb not in _PROGRAMS:
        _PROGRAMS[apply_wb] = build_program(apply_wb)
    return _PROGRAMS[apply_wb]


def kernel(F, P, norm_weight, norm_bias):
    from concourse.bass_utils import run_bass_kernel_spmd
    w = np.asarray(norm_weight, np.float32)
    b = np.asarray(norm_bias, np.float32)
    apply_wb = not (np.all(w == 1.0) and np.all(b == 0.0))
    nc = _program(apply_wb)
    maps = make_inmaps(F, P, norm_weight, norm_bias)
    res = run_bass_kernel_spmd(nc, maps, core_ids=list(range(8)), trace=False)
    return assemble(res.results)


# revision 7
# speedup vs baseline: 1.3885x; 1.0061x over previous
"""Trainium2 Bass kernel for nn_BRC_62715112457019 (sparse_attention), v2.

Head-parallel across 8 cores (core c = head c, both samples). Pixel-major
phase A (per-pixel stats/masks live on partitions -> no broadcast DMAs, tiny
128-wide DVE ops), fp8 DoubleRow phase B (QK^T and AV at 2 fp8 MACs/cycle),
per-qchunk transposed epilogue (no row-broadcasts), overlapped channel-attn
path and output writeback.

Pixel blocking: block b in [0,18) covers pixels [128b, 128b+128). Pixel-major
tiles are [128, 18*K] with column group b. Channel-major tensors ([8|16, HW])
are produced/consumed via PE transposes per block.
"""
import sys
for _p in ('/opt/trn_rl_repo', '/opt/pypackages'):
    if _p not in sys.path:
        sys.path.insert(0, _p)
import numpy as np
import ml_dtypes
from contextlib import ExitStack

import concourse.bass as bass
import concourse.bacc as bacc
import concourse.tile as tile
from concourse import mybir

dt = mybir.dt
F32 = dt.float32
BF16 = dt.bfloat16
FP8 = dt.float8e4
AF = mybir.ActivationFunctionType
OP = mybir.AluOpType
DR = mybir.MatmulPerfMode.DoubleRow

HW = 2304
NB = 18                       # 128-pixel blocks
CHUNKS = [(0, 512), (512, 512), (1024, 512), (1536, 512), (2048, 256)]
GRP = 2                       # logit blocks per exp group
BF = ml_dtypes.bfloat16
F8 = ml_dtypes.float8_e4m3fn


def host_constants(w8, b8):
    eye = np.eye(128, dtype=np.float32)
    eyeb = np.eye(128, dtype=BF)
    selsum = np.zeros((128, 5 * 37), BF)
    for c in range(5):
        selsum[0:64, 37 * c + c] = 1.0
        selsum[64:128, 37 * c + 32 + c] = 1.0
    wb16 = np.zeros((16, 2), np.float32)
    wb16[0:8, 0] = w8
    wb16[8:16, 0] = w8
    wb16[0:8, 1] = b8
    wb16[8:16, 1] = b8
    ones16 = np.ones((16, 1), np.float32)
    rep = np.zeros((6, 48), np.float32)
    for m in range(3):          # fg, bb, b
        for s in range(2):
            rep[2 * m + s, 16 * m + 8 * s:16 * m + 8 * s + 8] = 1.0
    return {"eye": eye, "eyeb": eyeb, "selsum": selsum, "wb16": wb16,
            "ones16": ones16, "rep": rep}


def make_inmaps(F, P, norm_weight, norm_bias):
    F = np.asarray(F, np.float32).reshape(2, 64, HW)
    P = np.asarray(P, np.float32).reshape(2, 48, 48)
    w = np.asarray(norm_weight, np.float32)
    b = np.asarray(norm_bias, np.float32)
    maps = []
    for c in range(8):
        m = host_constants(w[8 * c:8 * c + 8], b[8 * c:8 * c + 8])
        for n in range(2):
            m[f"Fb{n}"] = np.ascontiguousarray(F[n].astype(BF))
            m[f"F8_{n}"] = np.ascontiguousarray(F[n, 8 * c:8 * c + 8])
            m[f"P{n}"] = np.ascontiguousarray(P[n])
        maps.append(m)
    return maps


def assemble(results):
    out = np.empty((2, 64, 48, 48), np.float32)
    for c in range(8):
        for n in range(2):
            out[n, 8 * c:8 * c + 8] = results[c][f"out{n}"].reshape(8, 48, 48)
    return out


def build_program(apply_wb):
    nc = bacc.Bacc("TRN2", target_bir_lowering=False, debug=False)
    ins = {}
    for n in range(2):
        ins[f"Fb{n}"] = nc.dram_tensor(f"Fb{n}", [64, HW], BF16, kind="ExternalInput").ap()
        ins[f"F8_{n}"] = nc.dram_tensor(f"F8_{n}", [8, HW], F32, kind="ExternalInput").ap()
        ins[f"P{n}"] = nc.dram_tensor(f"P{n}", [48, 48], F32, kind="ExternalInput").ap()
    ins["eye"] = nc.dram_tensor("eye", [128, 128], F32, kind="ExternalInput").ap()
    ins["eyeb"] = nc.dram_tensor("eyeb", [128, 128], BF16, kind="ExternalInput").ap()
    ins["selsum"] = nc.dram_tensor("selsum", [128, 185], BF16, kind="ExternalInput").ap()
    ins["wb16"] = nc.dram_tensor("wb16", [16, 2], F32, kind="ExternalInput").ap()
    ins["ones16"] = nc.dram_tensor("ones16", [16, 1], F32, kind="ExternalInput").ap()
    ins["rep"] = nc.dram_tensor("rep", [6, 48], F32, kind="ExternalInput").ap()
    outs = [nc.dram_tensor(f"out{n}", [8, HW], F32, kind="ExternalOutput").ap() for n in range(2)]

    with tile.TileContext(nc) as tc:
        with ExitStack() as ctx:
            _body(ctx, tc, nc, ins, outs, apply_wb)
    nc.compile()
    return nc


# sobel slot indices (pairs of 50 cols: sample0|sample1, rows 0:48)
S_P50, S_PM, S_A1, S_TMP, S_B1, S_A1P, S_B1P, S_TCOL, S_GXT, S_GYT, S_M1, S_M2, \
    S_STT, S_BTM, S_BHW, S_FG, S_BG, S_BB = range(18)


def _body(ctx, tc, nc, ins, outs, apply_wb):
    pers = ctx.enter_context(tc.tile_pool(name="pers", bufs=1))
    sm = ctx.enter_context(tc.tile_pool(name="sm", bufs=1))

    # ---- persistent tiles ----
    eye = pers.tile([128, 128], F32, tag="eye")
    eyeb = pers.tile([128, 128], BF16, tag="eyeb")
    selsum = pers.tile([128, 185], BF16, tag="selsum")
    wb16 = pers.tile([16, 2], F32, tag="wb16")
    ones16 = pers.tile([16, 1], F32, tag="ones16")
    consts = pers.tile([128, 2], F32, tag="consts")     # col0 = eps
    F128 = pers.tile([128, HW], BF16, tag="F128")
    Fsq = pers.tile([128, HW], BF16, tag="Fsq")
    F16 = pers.tile([16, HW], F32, tag="F16")
    FnT = pers.tile([128, 288], F32, tag="FnT")         # 16b+8s+d
    qT = pers.tile([128, 288], BF16, tag="qT")
    NRM = pers.tile([128, 36], F32, tag="NRM")          # 2b+s
    RQB = pers.tile([128, 72], F32, tag="RQB")          # [0:36] sqrt, [36:72] recip
    MT = pers.tile([128, 296], F32, tag="MT")           # 74j+37t+32s+c
    MK3 = pers.tile([128, 18 * 48], F32, tag="MK3")     # 48b+{fg16,bb16,b16}
    mrows = pers.tile([6, HW], F32, tag="mrows")
    rep = pers.tile([6, 48], F32, tag="rep")
    qcm16 = pers.tile([16, HW], BF16, tag="qcm16")
    qcm1 = pers.tile([8, HW], BF16, tag="qcm1")
    trTav = [pers.tile([128, 288], FP8, tag=f"trTav{s}", name=f"trTav{s}") for s in range(2)]
    bfg24 = pers.tile([128, 864], F32, tag="bfg24")    # [b][s][fg8|bb8|Fn8]
    CM = [pers.tile([24, HW], BF16, tag=f"CM{s}", name=f"CM{s}") for s in range(2)]
    Sall = pers.tile([128, 2 * NB * 512], FP8, tag="Sall")
    w1 = pers.tile([128, 288], F32, tag="w1")
    spatT = pers.tile([128, 288], F32, tag="spatT")
    OUTT = pers.tile([128, 288], BF16, tag="OUTT")
    fin = [pers.tile([8, HW], F32, tag=f"fin{s}", name=f"fin{s}") for s in range(2)]
    rc = pers.tile([128, 8], F32, tag="rc")             # epilogue denominators
    sobm = pers.tile([48, 100 * 18], F32, tag="sobm")
    stm = pers.tile([37, 2560], F32, tag="stm")
    sq = pers.tile([128, 16], F32, tag="sq")
    # channel path smalls
    msk = pers.tile([16, 32], F32, tag="msk")
    r16f = pers.tile([16, 4], F32, tag="r16f")          # [0:2] sqrt, [2:4]=1/max(sqrt,..); col s
    rqd = pers.tile([8, 2], F32, tag="rqd")             # rq relocated to base 0
    A1 = pers.tile([8, 16], F32, tag="A1")
    A2 = pers.tile([8, 16], F32, tag="A2")
    expA = pers.tile([8, 16], F32, tag="expA")
    eden = pers.tile([8, 2], F32, tag="eden")
    rd8 = pers.tile([8, 2], F32, tag="rd8")
    rhs24T = pers.tile([8, 48], F32, tag="rhs24T")      # 24s col-block
    rhs24 = [pers.tile([24, 8], BF16, tag=f"rhs24_{s}", name=f"rhs24_{s}") for s in range(2)]
    WT = pers.tile([128, 16], F32, tag="WT") if apply_wb else None
    BT = pers.tile([128, 16], F32, tag="BT") if apply_wb else None

    def mu_col(b, s):
        return MT[:, 74 * (b % 4) + 32 * s + b // 4: 74 * (b % 4) + 32 * s + b // 4 + 1]

    def rs_col(b, s):
        o = 74 * (b % 4) + 37 + 32 * s + b // 4
        return MT[:, o:o + 1]

    mk3v = MK3[:].rearrange("p (b c) -> p b c", c=48)

    def mkv(m, s):
        return mk3v[:, :, 16 * m + 8 * s:16 * m + 8 * s + 8]

    fnv = FnT[:].rearrange("p (b c) -> p b c", c=16)
    w1v = w1[:].rearrange("p (b c) -> p b c", c=16)

    def fnsv(s):
        return fnv[:, :, 8 * s:8 * s + 8]

    def fnt(b, s):
        return FnT[:, 16 * b + 8 * s: 16 * b + 8 * s + 8]

    # =============== Phase A ===============
    with tc.tile_pool(name="psA", bufs=2, space="PSUM") as psA, \
         tc.tile_pool(name="psT", bufs=3, space="PSUM") as psT:

        nc.sync.dma_start(eye[:], ins["eye"])
        nc.sync.dma_start(eyeb[:], ins["eyeb"])
        nc.sync.dma_start(selsum[:], ins["selsum"])
        nc.sync.dma_start(wb16[:], ins["wb16"])
        nc.sync.dma_start(ones16[:], ins["ones16"])
        nc.sync.dma_start(rep[:], ins["rep"])
        nc.vector.memset(consts[:, 0:1], 1e-5)
        for s in range(2):
            nc.vector.memset(trTav[s][:], 0.0)
        for off, w in CHUNKS:
            nc.sync.dma_start(F128[0:64, off:off + w], ins["Fb0"][:, off:off + w])
            nc.sync.dma_start(F128[64:128, off:off + w], ins["Fb1"][:, off:off + w])
        nc.sync.dma_start(F16[0:8, :], ins["F8_0"])
        nc.sync.dma_start(F16[8:16, :], ins["F8_1"])

        # ---- masks (sobel in image space, batched samples in free dim) ----
        sv = sobm[:].rearrange("p (i s c) -> p i s c", s=2, c=50)

        def slot(i, r=(1, 49)):
            return sv[:, i, :, r[0]:r[1]]

        nc.gpsimd.memset(sobm[:, 0:200], 0.0)
        nc.sync.dma_start(slot(S_P50)[:, 0, :], ins["P0"])
        nc.sync.dma_start(slot(S_P50)[:, 1, :], ins["P1"])
        nc.scalar.activation(slot(S_PM), slot(S_P50), AF.Sigmoid)
        # exp table warmup (avoid mid-phase-B table load)
        nc.scalar.activation(sq[0:1, 0:1], consts[0:1, 0:1], AF.Exp)
        Pm0 = sv[:, S_PM]
        nc.vector.tensor_tensor(slot(S_A1), Pm0[:, :, 0:48], Pm0[:, :, 2:50], OP.subtract)
        nc.vector.tensor_tensor(slot(S_TMP), Pm0[:, :, 0:48], Pm0[:, :, 2:50], OP.add)
        nc.vector.scalar_tensor_tensor(slot(S_B1), Pm0[:, :, 1:49], 2.0, slot(S_TMP),
                                       OP.mult, OP.add)
        nc.gpsimd.memset(sobm[:, 100 * S_A1P:100 * S_A1P + 200], 0.0)
        for s in range(2):
            pt1 = psT.tile([128, 512], F32, tag="psT", name=f"pt1_{s}")
            nc.tensor.transpose(pt1[0:48, 0:48], slot(S_A1)[:, s, :], eye[0:48, 0:48])
            nc.vector.tensor_copy(slot(S_A1P)[:, s, :], pt1[0:48, 0:48])
            pt2 = psT.tile([128, 512], F32, tag="psT", name=f"pt2_{s}")
            nc.tensor.transpose(pt2[0:48, 0:48], slot(S_B1)[:, s, :], eye[0:48, 0:48])
            nc.vector.tensor_copy(slot(S_B1P)[:, s, :], pt2[0:48, 0:48])
        A1p = sv[:, S_A1P]
        B1p = sv[:, S_B1P]
        nc.vector.tensor_tensor(slot(S_TCOL), A1p[:, :, 0:48], A1p[:, :, 2:50], OP.add)
        nc.vector.scalar_tensor_tensor(slot(S_GXT), A1p[:, :, 1:49], 2.0, slot(S_TCOL),
                                       OP.mult, OP.add)
        nc.vector.tensor_tensor(slot(S_GYT), B1p[:, :, 0:48], B1p[:, :, 2:50], OP.subtract)
        nc.vector.tensor_tensor(slot(S_M1), slot(S_GXT), slot(S_GXT), OP.mult)
        nc.vector.tensor_tensor(slot(S_M2), slot(S_GYT), slot(S_GYT), OP.mult)
        nc.vector.tensor_tensor(slot(S_STT), slot(S_M1), slot(S_M2), OP.add)
        nc.vector.tensor_scalar(slot(S_BTM), slot(S_STT), 0.0, None, OP.is_gt)
        for s in range(2):
            pt3 = psT.tile([128, 512], F32, tag="psT", name=f"pt3_{s}")
            nc.tensor.transpose(pt3[0:48, 0:48], slot(S_BTM)[:, s, :], eye[0:48, 0:48])
            nc.vector.tensor_copy(slot(S_BHW)[:, s, :], pt3[0:48, 0:48])
        nc.vector.tensor_scalar(slot(S_FG), slot(S_P50), 0.0, None, OP.is_gt)
        nc.vector.tensor_scalar(slot(S_BG), slot(S_P50), 0.0, None, OP.is_lt)
        nc.vector.scalar_tensor_tensor(slot(S_BB), slot(S_BG), 1.0, slot(S_BHW),
                                       OP.mult, OP.max)
        # flatten masks to rows, then per-block transposes -> MKT
        for m, si in ((0, S_FG), (2, S_BB), (4, S_BHW)):
            for s in range(2):
                nc.sync.dma_start(mrows[m + s:m + s + 1, :], slot(si)[:, s, :])
        for b in range(NB):
            pm = psT.tile([128, 512], F32, tag="psT", name=f"pm{b}")
            nc.tensor.matmul(pm[0:128, 0:48], mrows[:, 128 * b:128 * b + 128],
                             rep[:], start=True, stop=True)
            nc.vector.tensor_copy(MK3[:, 48 * b:48 * b + 48], pm[0:128, 0:48])

        # ---- LayerNorm stats (channel-major PE reduction) ----
        for off, w in CHUNKS:
            nc.vector.tensor_tensor(Fsq[:, off:off + w], F128[:, off:off + w],
                                    F128[:, off:off + w], OP.mult)
        psumsA = psA.tile([128, 512], F32, tag="psA", name="psumsA")
        psumsB = psA.tile([128, 512], F32, tag="psA", name="psumsB")
        for c, (off, w) in enumerate(CHUNKS):
            nc.tensor.matmul(psumsA[0:37, 0:w], selsum[:, 37 * c:37 * c + 37],
                             F128[:, off:off + w], start=(c == 0), stop=(c == 4))
            nc.tensor.matmul(psumsB[0:37, 0:w], selsum[:, 37 * c:37 * c + 37],
                             Fsq[:, off:off + w], start=(c == 0), stop=(c == 4))
        s2 = stm[:, 0:512]
        varT = stm[:, 512:1024]
        sd = stm[:, 1024:1536]
        rstd = stm[:, 1536:2048]
        mu = stm[:, 2048:2560]
        nc.scalar.activation(s2, psumsA[0:37, :], AF.Square, scale=0.125)
        nc.vector.scalar_tensor_tensor(varT, psumsB[0:37, :], 1.0, s2, OP.mult, OP.subtract)
        nc.scalar.activation(sd, varT, AF.Sqrt, bias=consts[0:37, 0:1], scale=1.0 / 64.0)
        nc.vector.reciprocal(rstd, sd)
        nc.vector.tensor_scalar(mu, psumsA[0:37, :], 1.0 / 64.0, None, OP.mult)
        # stats -> pixel-major MT via transposes
        for j in range(4):
            for t, src in ((0, mu), (1, rstd)):
                pst = psT.tile([128, 512], F32, tag="psT", name=f"pst{j}_{t}")
                nc.tensor.transpose(pst[0:128, 0:37], src[:, 128 * j:128 * j + 128],
                                    eye[0:37, 0:37])
                nc.vector.tensor_copy(MT[:, 74 * j + 37 * t:74 * j + 37 * t + 37],
                                      pst[0:128, 0:37])

        if apply_wb:
            # broadcast per-channel w/b across partitions via ones-matmul
            pw = psT.tile([128, 512], F32, tag="psT", name="pw")
            nc.tensor.transpose(pw[0:2, 0:16], wb16[:], eye[0:16, 0:16])
            wbrow = sm.tile([2, 16], F32, tag="wbrow")
            nc.vector.tensor_copy(wbrow[:], pw[0:2, 0:16])
            onesr = sm.tile([1, 128], F32, tag="onesr")
            nc.vector.memset(onesr[:], 1.0)
            pw2 = psT.tile([128, 512], F32, tag="psT", name="pw2")
            nc.tensor.matmul(pw2[0:128, 0:16], onesr[:], wbrow[0:1, :], start=True, stop=True)
            nc.vector.tensor_copy(WT[:], pw2[0:128, 0:16])
            pw3 = psT.tile([128, 512], F32, tag="psT", name="pw3")
            nc.tensor.matmul(pw3[0:128, 0:16], onesr[:], wbrow[1:2, :], start=True, stop=True)
            nc.vector.tensor_copy(BT[:], pw3[0:128, 0:16])

        # ---- per-block: Fn_T, norms, q_T ----
        for b in range(NB):
            pF = psT.tile([128, 512], F32, tag="psT", name=f"pF{b}")
            nc.tensor.transpose(pF[0:128, 0:16], F16[:, 128 * b:128 * b + 128],
                                eye[0:16, 0:16])
            for s in range(2):
                nc.vector.tensor_scalar(fnt(b, s), pF[0:128, 8 * s:8 * s + 8],
                                        mu_col(b, s), rs_col(b, s), OP.subtract, OP.mult)
            if apply_wb:
                nc.vector.tensor_tensor(FnT[:, 16 * b:16 * b + 16],
                                        FnT[:, 16 * b:16 * b + 16], WT[:], OP.mult)
                nc.vector.tensor_tensor(FnT[:, 16 * b:16 * b + 16],
                                        FnT[:, 16 * b:16 * b + 16], BT[:], OP.add)
            for s in range(2):
                nc.vector.scalar_tensor_tensor(sq[:, 8 * s:8 * s + 8], fnt(b, s), 1.0,
                                               fnt(b, s), OP.mult, OP.mult,
                                               accum_out=NRM[:, 2 * b + s:2 * b + s + 1])
        nc.scalar.activation(RQB[:, 0:36], NRM[:], AF.Sqrt)
        nc.vector.tensor_scalar(RQB[:, 0:36], RQB[:, 0:36], 1e-12, None, OP.max)
        nc.vector.reciprocal(RQB[:, 36:72], RQB[:, 0:36])
        for b in range(NB):
            for s in range(2):
                nc.vector.tensor_scalar(qT[:, 16 * b + 8 * s:16 * b + 8 * s + 8],
                                        fnt(b, s), RQB[:, 36 + 2 * b + s:37 + 2 * b + s],
                                        None, OP.mult)
            pQ = psT.tile([128, 512], F32, tag="psT", name=f"pQ{b}")
            pQb = pQ[0:16, 0:64].bitcast(BF16)
            nc.tensor.transpose(pQb, qT[:, 16 * b:16 * b + 16], eyeb[:, :])
            nc.scalar.activation(qcm16[:, 128 * b:128 * b + 128], pQb, AF.Copy)
        nc.sync.dma_start(qcm1[:], qcm16[8:16, :])

        # ---- masked features: batched full-tile ops over 3D views ----
        b24 = bfg24[:].rearrange("p (b s c) -> p b s c", s=2, c=24)
        tvv = [trTav[s][:].rearrange("p (b c) -> p b c", c=16) for s in range(2)]
        for s in range(2):
            nc.vector.tensor_tensor(tvv[s][:, :, 0:8], fnsv(s), mkv(0, s), OP.mult)
            nc.vector.tensor_copy(tvv[s][:, :, 8:9], mk3v[:, :, 8 * s:8 * s + 1])
            nc.vector.tensor_tensor(b24[:, :, s, 0:8], fnsv(s), mkv(0, s), OP.mult)
            nc.vector.tensor_tensor(b24[:, :, s, 8:16], fnsv(s), mkv(1, s), OP.mult)
            nc.vector.tensor_copy(b24[:, :, s, 16:24], fnsv(s))
        for b in range(NB):
            for s in range(2):
                pC = psT.tile([128, 512], F32, tag="psT", name=f"pC{b}_{s}")
                nc.tensor.transpose(pC[0:24, 0:128],
                                    bfg24[:, 48 * b + 24 * s:48 * b + 24 * s + 24],
                                    eye[:, :])
                nc.vector.tensor_copy(CM[s][:, 128 * b:128 * b + 128],
                                      pC[0:24, 0:128])
        # w1 = Fn + b*(q - Fn)   (final out = w1 + Fn + b*spat + Fch + qc)
        nc.vector.tensor_tensor(w1[:], qT[:], FnT[:], OP.subtract)
        for s in range(2):
            nc.vector.tensor_tensor(w1v[:, :, 8 * s:8 * s + 8], w1v[:, :, 8 * s:8 * s + 8],
                                    mkv(2, s), OP.mult)
        nc.vector.tensor_tensor(w1[:], w1[:], FnT[:], OP.add)

    # =============== Phase B + channel path + finals ===============
    Sv = Sall[:].rearrange("p (t b x) -> p t b x", t=2, b=NB)
    tv = [trTav[s][:].rearrange("p (pb i c) -> p pb i c", i=2, c=16) for s in range(2)]

    with tc.tile_pool(name="psL", bufs=2, space="PSUM") as psL, \
         tc.tile_pool(name="psO", bufs=2, space="PSUM") as psO, \
         tc.tile_pool(name="psM", bufs=1, space="PSUM") as psMp, \
         tc.tile_pool(name="sS", bufs=2) as sS:
        psM = psMp.tile([128, 512], F32, tag="psM")
        kslot = [0]

        def phase_b(s):
            qsrc = qcm16[0:8, :] if s == 0 else qcm1[:]
            for jc, (joff, jw) in enumerate(CHUNKS):
                Sb = Sv[:, jc % 2]
                psOt = psO.tile([128, 512], F32, tag="psO", name=f"psO{s}_{jc}")

                def logits(g):
                    Lg = psL.tile([128, GRP * 512], F32, tag="L", name=f"L{s}_{jc}_{g}")
                    Lv = Lg[:].rearrange("p (i x) -> p i x", i=GRP)
                    for i in range(GRP):
                        b = GRP * g + i
                        nc.tensor.matmul(Lv[:, i, 0:jw], qsrc[:, 128 * b:128 * b + 128],
                                         qsrc[:, joff:joff + jw],
                                         start=True, stop=True)
                    nc.scalar.activation(Sb[:, GRP * g:GRP * g + GRP, 0:jw],
                                         Lv[:, :, 0:jw], AF.Exp)

                def av(pb):
                    nc.tensor.matmul(psOt[0:16, 0:jw], tv[s][:, pb], Sb[:, 2 * pb:2 * pb + 2, 0:jw],
                                     start=(pb == 0), stop=(pb == 8), perf_mode=DR)

                logits(0)
                for g in range(1, 9):
                    logits(g)
                    av(g - 1)
                av(8)
                # transposed epilogue: spat = num/den, pixel-major
                avs = sS.tile([9, 512], F32, tag="avs", name=f"avs{s}_{jc}")
                nc.vector.tensor_copy(avs[:, 0:jw], psOt[0:9, 0:jw])
                for j in range(jw // 128):
                    b = 4 * jc + j
                    k = kslot[0] % 8
                    kslot[0] += 1
                    pslot = psM[0:128, 9 * k:9 * k + 9]
                    nc.tensor.transpose(pslot, avs[:, 128 * j:128 * j + 128], eye[0:9, 0:9])
                    nc.vector.reciprocal(rc[:, k:k + 1], pslot[:, 8:9])
                    nc.vector.tensor_scalar(spatT[:, 16 * b + 8 * s:16 * b + 8 * s + 8],
                                            pslot[:, 0:8], rc[:, k:k + 1], None, OP.mult)
                finals_chunk(s, jc, joff, jw)

        def channel_path():
            # per-sample Gram accumulation: psum16 = [fg|bb].T @ [fg|bb] over pixels
            ps16 = [psM[0:16, 72 + 16 * s:72 + 16 * s + 16] for s in range(2)]
            for s in range(2):
                for b in range(NB):
                    ap = bfg24[:, 48 * b + 24 * s:48 * b + 24 * s + 16]
                    nc.tensor.matmul(ps16[s], ap, ap,
                                     start=(b == 0), stop=(b == NB - 1))
            for s in range(2):
                nc.vector.tensor_tensor(msk[:, 16 * s:16 * s + 16], ps16[s],
                                        eye[0:16, 0:16], OP.mult)
                pd = psM[0:16, 104 + 2 * s:104 + 2 * s + 1]
                nc.tensor.matmul(pd, msk[:, 16 * s:16 * s + 16], ones16[:],
                                 start=True, stop=True)
                nc.scalar.activation(r16f[:, s:s + 1], pd, AF.Sqrt)
            nc.vector.tensor_scalar(r16f[:, 0:2], r16f[:, 0:2], 1e-12, None, OP.max)
            nc.vector.reciprocal(r16f[:, 2:4], r16f[:, 0:2])
            # rq (rows 8:16 of r16f) relocated to base 0
            nc.sync.dma_start(rqd[:], r16f[8:16, 2:4])
            for s in range(2):
                # Gram is symmetric: G^T[k, q] = Gram[0:8, 8:16] (fg rows, bb cols)
                nc.vector.tensor_scalar(A1[:, 8 * s:8 * s + 8],
                                        ps16[s][0:8, 8:16],
                                        r16f[0:8, 2 + s:3 + s], None, OP.mult)
                pA = psM[0:8, 140 + 8 * s:140 + 8 * s + 8]
                nc.tensor.transpose(pA, A1[:, 8 * s:8 * s + 8], eye[0:8, 0:8])
                nc.vector.tensor_scalar(A2[:, 8 * s:8 * s + 8], pA, rqd[:, s:s + 1],
                                        None, OP.mult)
                nc.scalar.activation(expA[:, 8 * s:8 * s + 8], A2[:, 8 * s:8 * s + 8],
                                     AF.Exp, accum_out=eden[:, s:s + 1])
                nc.vector.reciprocal(rd8[:, s:s + 1], eden[:, s:s + 1])
                # rhs24T[q, :] = [attn_c[q, k], diag(rq)[q, k], I8[q, d]] -> transpose
                base = 24 * s
                nc.vector.tensor_scalar(rhs24T[:, base:base + 8], expA[:, 8 * s:8 * s + 8],
                                        rd8[:, s:s + 1], None, OP.mult)
                nc.vector.tensor_scalar(rhs24T[:, base + 8:base + 16], eye[0:8, 0:8],
                                        rqd[:, s:s + 1], None, OP.mult)
                nc.vector.tensor_copy(rhs24T[:, base + 16:base + 24], eye[0:8, 0:8])
                pR = psM[0:24, 156 + 8 * s:156 + 8 * s + 8]
                nc.tensor.transpose(pR, rhs24T[:, base:base + 24], eye[0:8, 0:8])
                nc.vector.tensor_copy(rhs24[s][:], pR)

        spv = spatT[:].rearrange("p (b c) -> p b c", c=16)
        ov = OUTT[:].rearrange("p (b c) -> p b c", c=16)

        def finals_chunk(s, jc, joff, jw):
            nb = jw // 128
            bs = slice(4 * jc, 4 * jc + nb)
            nc.vector.tensor_tensor(ov[:, bs, 8 * s:8 * s + 8], spv[:, bs, 8 * s:8 * s + 8],
                                    mkv(2, s)[:, bs, :], OP.mult)
            nc.vector.tensor_tensor(ov[:, bs, 8 * s:8 * s + 8], ov[:, bs, 8 * s:8 * s + 8],
                                    w1v[:, bs, 8 * s:8 * s + 8], OP.add)
            for b in range(4 * jc, 4 * jc + nb):
                fslot = psM[0:128, 172 + 8 * (b % 2):172 + 8 * (b % 2) + 8]
                nc.tensor.matmul(fslot, CM[s][:, 128 * b:128 * b + 128], rhs24[s][:],
                                 start=True, stop=True)
                o = OUTT[:, 16 * b + 8 * s:16 * b + 8 * s + 8]
                nc.vector.tensor_tensor(o, o, fslot, OP.add)
                tslot = psM[0:8, 192 + 64 * (b % 2):192 + 64 * (b % 2) + 64].bitcast(BF16)
                nc.tensor.transpose(tslot, o, eyeb[:, :])
                nc.vector.tensor_copy(fin[s][:, 128 * b:128 * b + 128], tslot)
            nc.sync.dma_start(outs[s][:, joff:joff + jw], fin[s][:, joff:joff + jw])

        channel_path()
        phase_b(0)
        phase_b(1)


_PROGRAMS = {}


def _program(apply_wb=False):
    if apply_wb not in _PROGRAMS:
        _PROGRAMS[apply_wb] = build_program(apply_wb)
    return _PROGRAMS[apply_wb]


def kernel(F, P, norm_weight, norm_bias):
    from concourse.bass_utils import run_bass_kernel_spmd
    w = np.asarray(norm_weight, np.float32)
    b = np.asarray(norm_bias, np.float32)
    apply_wb = not (np.all(w == 1.0) and np.all(b == 0.0))
    nc = _program(apply_wb)
    maps = make_inmaps(F, P, norm_weight, norm_bias)
    res = run_bass_kernel_spmd(nc, maps, core_ids=list(range(8)), trace=False)
    return assemble(res.results)
